# revision 2
# baseline (speedup 1.0000x reference)
"""GCN message-passing kernel for Trainium2 (8 NeuronCores, SPMD) — v2.

Math (matches the reference):
    gf   = RF @ W_g                          (2048, 3)   gate features
    H_k  = RF @ W_k                          (2048, 4096) per edge type k in {0,1,2}
    gate(e) = sigmoid(gf[src_e, k_e] + b_glab[p_e])
    upd[t]  = sum_{e->t} gate(e) * (H_{k_e}[src_e] + b_lab[p_e])
    out  = relu(upd)

v2 key idea (on top of the v1 block-diagonal gate-matrix formulation):
only ~47% of regions ever appear as a message SOURCE for edge types 0/1
(unique objs/subjs per image ~15 of 32).  H_0/H_1 rows for unused
sources are dead work.  We therefore compute H_0/H_1 only for the
PACKED list of unique sources: the used RF rows are compacted (host
side) into ~8 pseudo-blocks of 128 per edge type instead of 16 full
blocks, cutting the dominant PE matmul stream from 48 to ~32
block-passes (~2/3 of v1's tensor-engine work).  The self-loop type
k=2 still needs every region, and the per-region gate features gf ride
along the k=2 pass as interleaved 3-column matmuls.

Aggregation stays on device: per target block, block-local gate
matrices in PACKED source coordinates are built from one-hot index
matrices (host prepares 0/1 matrices from rels; gates themselves are
computed on Trainium).  A target block's sources span at most 2
pseudo-blocks (sources are packed in region order), so stage3 needs at
most 2 matmuls per edge type.

Sharding: unchanged from v1 — output D dim split 8 ways; each core
computes all 2048 rows x its 512 columns.  No collectives.

The pseudo-block structure depends on `rels`, so the program is
compiled per relation pattern (cached by rels bytes).
"""

import numpy as np
import ml_dtypes

# problem constants (hardcoded per contract)
N_IMG = 64
REG = 32
RPI = 32
NUM_REL = 20
D = 4096
NPRED = 81
N = N_IMG * REG          # 2048
NCORES = 8
CW = D // NCORES         # 512 output cols per core
NBLK = N // 128          # 16 row blocks
IPB = 128 // REG         # 4 images per block
EPB = IPB * NUM_REL      # 80 edges per block per edge type

BF = ml_dtypes.bfloat16
F8 = ml_dtypes.float8_e4m3fn

import os
FP8K0 = os.environ.get("FP8K0", "0") == "1"  # probe: k0 packed passes in fp8
S_R = 32.0     # fp8 scale for RF
S_W = 2048.0   # fp8 scale for W

_prog_cache = {}


def _to_f8(x):
    return np.clip(x, -440.0, 440.0).astype(F8)


def _plan_packing(rels):
    """Host-side packing plan from the relation list.

    Returns a dict with, per edge type k in (0, 1):
      u[k]        sorted unique global source region ids
      P[k]        number of 128-wide pseudo-blocks
      pos[k]      region id -> packed position (array of len N, -1 unused)
    and per (block b, k):
      pb_lo/pb_hi pseudo-blocks containing this block's sources
      cross       whether pb_hi > pb_lo
    plus cross-index assignment for the compact hi-side one-hot tensor.
    """
    rels_r = np.asarray(rels).reshape(N_IMG, RPI, 3)[:, :NUM_REL].reshape(-1, 3)
    s, o = rels_r[:, 1], rels_r[:, 2]
    srcs = {0: o, 1: s}
    plan = {"u": {}, "P": {}, "pos": {}, "pb_lo": {}, "pb_hi": {},
            "cross_idx": {}, "n_cross": 0, "last_user": {}}
    for k in (0, 1):
        u = np.unique(srcs[k])
        pos = np.full(N, -1, np.int64)
        pos[u] = np.arange(len(u))
        plan["u"][k] = u
        plan["P"][k] = -(-len(u) // 128)
        plan["pos"][k] = pos
    n_cross = 0
    for b in range(NBLK):
        for k in (0, 1):
            pk = plan["pos"][k][srcs[k][b * EPB:(b + 1) * EPB]]
            assert np.all(pk >= 0)
            pbs = pk // 128
            lo, hi = int(pbs.min()), int(pbs.max())
            assert hi - lo <= 1, f"block {b} k {k} spans {hi - lo + 1} pbs"
            plan["pb_lo"][(b, k)] = lo
            plan["pb_hi"][(b, k)] = hi
            if hi > lo:
                plan["cross_idx"][(b, k)] = n_cross
                n_cross += 1
            for pb in (lo, hi):
                key = (k, pb)
                plan["last_user"][key] = max(plan["last_user"].get(key, -1), b)
    plan["n_cross"] = max(n_cross, 1)
    return plan


def _plan_key(plan):
    items = [tuple(sorted(plan["u"][k].tolist())) for k in (0, 1)]
    items.append(tuple(sorted(plan["pb_lo"].items())))
    items.append(tuple(sorted(plan["pb_hi"].items())))
    return hash(tuple(map(tuple, map(str, items))))


def _build_program(plan):
    import concourse.bass as bass
    import concourse.tile as tile
    from concourse import bacc, mybir

    bf16 = mybir.dt.bfloat16
    f32 = mybir.dt.float32
    AF = mybir.ActivationFunctionType
    ALU = mybir.AluOpType

    P0, P1 = plan["P"][0], plan["P"][1]
    NCROSS = plan["n_cross"]

    nc = bacc.Bacc("TRN2", target_bir_lowering=False, debug=False,
                   num_devices=NCORES)

    f8e4 = mybir.dt.float8e4

    rft = nc.dram_tensor("rft", [NBLK, 128, 32 * 128], bf16, kind="ExternalInput").ap()
    prft = nc.dram_tensor("prft", [P0 + P1, 128, 32 * 128], bf16, kind="ExternalInput").ap()
    if FP8K0:
        prft8 = nc.dram_tensor("prft8", [P0, 128, 32 * 128], f8e4,
                               kind="ExternalInput").ap()
        w08 = nc.dram_tensor("w08", [128, 32 * CW], f8e4,
                             kind="ExternalInput").ap()
    # w layout: k-order (2, 0, 1), i.e. cols [k2 | k0 | k1], chunk-major inside
    w = nc.dram_tensor("w", [128, 3 * 32 * CW], bf16, kind="ExternalInput").ap()
    wg = nc.dram_tensor("wg", [128, 32 * 3], bf16, kind="ExternalInput").ap()
    blab = nc.dram_tensor("blab", [NPRED, CW], bf16, kind="ExternalInput").ap()
    bgb = nc.dram_tensor("bgb", [128, NPRED], bf16, kind="ExternalInput").ap()
    srct = nc.dram_tensor("srct", [128, NBLK * 2 * EPB], bf16, kind="ExternalInput").ap()
    srcoL = nc.dram_tensor("srcoL", [EPB, NBLK * 2 * 128], bf16, kind="ExternalInput").ap()
    srcoH = nc.dram_tensor("srcoH", [EPB, NCROSS * 128], bf16, kind="ExternalInput").ap()
    tgto = nc.dram_tensor("tgto", [EPB, NBLK * 2 * 128], bf16, kind="ExternalInput").ap()
    p1h = nc.dram_tensor("p1h", [EPB, NBLK * NPRED], bf16, kind="ExternalInput").ap()
    p1hs = nc.dram_tensor("p1hs", [128, NPRED], bf16, kind="ExternalInput").ap()
    ident = nc.dram_tensor("ident", [128, 128], bf16, kind="ExternalInput").ap()
    out = nc.dram_tensor("out", [NBLK, 128, CW], bf16, kind="ExternalOutput").ap()

    DEPTH = 4  # k2 blocks run before the first packed pass (absorbs W DMA)
    with tile.TileContext(nc) as tc:
        with (
            tc.tile_pool(name="consts", bufs=1) as cpool,
            tc.tile_pool(name="rft", bufs=4) as rpool,
            tc.tile_pool(name="prft", bufs=2) as ppool,
            tc.tile_pool(name="hp", bufs=5) as hppool,
            tc.tile_pool(name="deep", bufs=8) as dpool,
            tc.tile_pool(name="small", bufs=2) as spool,
            tc.tile_pool(name="osb", bufs=2) as opool,
            tc.tile_pool(name="ph", bufs=2, space="PSUM") as php,
            tc.tile_pool(name="pgf", bufs=1, space="PSUM") as pgfp,
            tc.tile_pool(name="prg", bufs=1, space="PSUM") as prgp,
            tc.tile_pool(name="pgt", bufs=1, space="PSUM") as pgtp,
            tc.tile_pool(name="pmt", bufs=2, space="PSUM") as pmtp,
            tc.tile_pool(name="pout", bufs=1, space="PSUM") as poutp,
        ):
            # --- input DMAs, ordered by when the PE needs each tensor ---
            wg_sb = cpool.tile([128, 32 * 3], bf16, tag="wg")
            nc.sync.dma_start(out=wg_sb[:], in_=wg[:])
            WCH = 4 * CW  # w chunk: 4 d-tiles
            w_ch = {k: [cpool.tile([128, WCH], bf16, tag=f"w{k}c{i}",
                                   name=f"w{k}c{i}")
                        for i in range(8)]
                    for k in ((2, 1) if FP8K0 else (2, 0, 1))}

            def _rft_half(b, h):
                t = rpool.tile([128, 16 * 128], bf16, tag=f"rft{h}",
                               name=f"rft{h}_{b}")
                nc.sync.dma_start(out=t[:],
                                  in_=rft[b, :, h * 16 * 128:(h + 1) * 16 * 128])
                return t

            def _load_rft(b):
                rft_tiles[b] = [_rft_half(b, 0), _rft_half(b, 1)]

            def _prft_half(i, h):
                t = ppool.tile([128, 16 * 128], bf16, tag=f"prft{h}",
                               name=f"prft{h}_{i}")
                nc.sync.dma_start(out=t[:],
                                  in_=prft[i, :, h * 16 * 128:(h + 1) * 16 * 128])
                return t

            def _prft8_half(i, h):
                t = ppool.tile([128, 16, 128], f8e4, tag=f"p8_{h}",
                               name=f"p8_{h}_{i}")
                nc.sync.dma_start(out=t[:],
                                  in_=prft8[i, :, h * 16 * 128:(h + 1) * 16 * 128])
                return t

            def _load_prft(k, pb):
                if FP8K0 and k == 0:
                    prft_tiles[(k, pb)] = [_prft8_half(pb, 0), _prft8_half(pb, 1)]
                else:
                    i = pb if k == 0 else P0 + pb
                    prft_tiles[(k, pb)] = [_prft_half(i, 0), _prft_half(i, 1)]

            if FP8K0:
                w08_sb = cpool.tile([128, 32, CW], f8e4, tag="w08",
                                    name="w08")

            def _load_w(k):
                if FP8K0 and k == 0:
                    nc.sync.dma_start(out=w08_sb[:], in_=w08[:])
                    return
                # dram w layout is k-order (2, 0, 1), chunk-major
                base = {2: 0, 0: 1, 1: 2}[k] * 32 * CW
                for i in range(8):
                    nc.sync.dma_start(out=w_ch[k][i][:],
                                      in_=w[:, base + i * WCH:
                                            base + (i + 1) * WCH])

            rft_tiles, prft_tiles = {}, {}
            nc.sync.dma_start(out=w_ch[2][0][:], in_=w[:, 0:WCH])
            _load_rft(0)
            for i in range(1, 8):
                nc.sync.dma_start(out=w_ch[2][i][:],
                                  in_=w[:, i * WCH:(i + 1) * WCH])
            # small consts needed by build(0) early
            blab_sb = cpool.tile([NPRED, CW], bf16, tag="blab")
            nc.sync.dma_start(out=blab_sb[:], in_=blab[:])
            bgb_sb = cpool.tile([128, NPRED], bf16, tag="bgb")
            nc.sync.dma_start(out=bgb_sb[:], in_=bgb[:])
            p1hs_sb = cpool.tile([128, NPRED], bf16, tag="p1hs")
            nc.sync.dma_start(out=p1hs_sb[:], in_=p1hs[:])
            ident_sb = cpool.tile([128, 128], bf16, tag="ident")
            nc.sync.dma_start(out=ident_sb[:], in_=ident[:])
            _load_rft(1)
            srct_sb = cpool.tile([128, NBLK * 2 * EPB], bf16, tag="srct")
            nc.sync.dma_start(out=srct_sb[:], in_=srct[:])
            p1h_sb = cpool.tile([EPB, NBLK * NPRED], bf16, tag="p1h")
            nc.sync.dma_start(out=p1h_sb[:], in_=p1h[:])
            _load_rft(2)
            srcoL_sb = cpool.tile([EPB, NBLK * 2 * 128], bf16, tag="srcoL")
            nc.sync.dma_start(out=srcoL_sb[:], in_=srcoL[:])
            srcoH_sb = cpool.tile([EPB, NCROSS * 128], bf16, tag="srcoH")
            nc.sync.dma_start(out=srcoH_sb[:], in_=srcoH[:])
            tgto_sb = cpool.tile([EPB, NBLK * 2 * 128], bf16, tag="tgto")
            nc.sync.dma_start(out=tgto_sb[:], in_=tgto[:])
            _load_rft(3)
            _load_prft(0, 0)
            _load_w(0)
            _load_prft(1, 0)
            _load_w(1)

            h2_sb, hp_sb, gf_tiles, g2_tiles, mtgt = {}, {}, {}, {}, {}

            def rft_lhsT(b, d):
                return rft_tiles[b][d // 16][:, (d % 16) * 128:(d % 16 + 1) * 128]

            def prft_lhsT(k, pb, d):
                return prft_tiles[(k, pb)][d // 16][:, (d % 16) * 128:(d % 16 + 1) * 128]

            from concourse.tile_rust import add_dep_helper

            def k2gf_pass(b):
                """H_2(b) = RF_b @ W_2 with gf(b) interleaved (gf matmuls
                reuse the H matmul's stationary operand via ldweights=False;
                the local ordering chain keeps each gf adjacent to its H
                partner).  The self-loop gate is folded into the PSUM->SBUF
                copy: h2s = diag(g2) @ H_2."""
                ph_t = php.tile([128, CW], f32, tag="ph", name=f"ph{b}_2")
                pgf_t = pgfp.tile([128, 3], f32, tag="pgf", name=f"pgf{b}")
                prev = None
                for d in range(32):
                    lhsT = rft_lhsT(b, d)
                    nc.tensor.matmul(ph_t[:], lhsT,
                                     w_ch[2][d // 4][:, (d % 4) * CW:(d % 4 + 1) * CW],
                                     start=(d == 0), stop=(d == 31))
                    h_inst = nc.main_func.blocks[-1].instructions[-1]
                    assert h_inst.opcode == "Matmult"
                    if prev is not None:
                        add_dep_helper(h_inst, prev, sync=False,
                                       reason="k2-chain")
                    nc.tensor.matmul(pgf_t[:], lhsT,
                                     wg_sb[:, d * 3:(d + 1) * 3],
                                     start=(d == 0), stop=(d == 31))
                    gf_inst = nc.main_func.blocks[-1].instructions[-1]
                    assert gf_inst.opcode == "Matmult"
                    gf_inst.ldweights = False
                    add_dep_helper(gf_inst, h_inst, sync=False,
                                   reason="k2-pair")
                    prev = gf_inst
                gf_sb = dpool.tile([128, 3], f32, tag="gf", name=f"gf{b}")
                nc.vector.tensor_copy(out=gf_sb[:], in_=pgf_t[:])
                gf_tiles[b] = gf_sb
                g2 = dpool.tile([128, 1], f32, tag="g2", name=f"g2_{b}")
                nc.scalar.activation(g2[:], bgb_sb[:, 0:1], AF.Sigmoid,
                                     bias=gf_sb[:, 2:3])
                g2_tiles[b] = g2
                hk = dpool.tile([128, CW], bf16, tag="h2", name=f"h2_{b}")
                nc.vector.tensor_scalar_mul(hk[:], ph_t[:], g2[:])
                h2_sb[b] = hk

            def ppass(k, pb):
                """Packed H_k rows for pseudo-block pb."""
                ph_t = php.tile([128, CW], f32, tag="ph", name=f"php{k}_{pb}")
                if FP8K0 and k == 0:
                    # fp8 DoubleRow: chunk-pair (K=256) per matmul; lhsT free
                    # [2,128] and rhs free [2,512] carry an explicit pair dim
                    for dp in range(16):
                        half = prft_tiles[(k, pb)][dp // 8]
                        j = dp % 8
                        lhsT = half[:, 2 * j:2 * j + 2, :]
                        rhs = w08_sb[:, 2 * dp:2 * dp + 2, :]
                        nc.tensor.matmul(
                            ph_t[:], lhsT, rhs, start=(dp == 0),
                            stop=(dp == 15),
                            perf_mode=mybir.MatmulPerfMode.DoubleRow)
                else:
                    for d in range(32):
                        nc.tensor.matmul(ph_t[:], prft_lhsT(k, pb, d),
                                         w_ch[k][d // 4][:, (d % 4) * CW:(d % 4 + 1) * CW],
                                         start=(d == 0), stop=(d == 31))
                hk = hppool.tile([128, CW], bf16, tag=f"hp{k}",
                                 name=f"hp{k}_{pb}")
                nc.vector.tensor_copy(out=hk[:], in_=ph_t[:])
                hp_sb[(k, pb)] = hk
                del prft_tiles[(k, pb)]

            def build(b):
                """Gates -> packed-coordinate gate matrices (lhsT form,
                [packed_pos_in_pb, target]) and G^T for the b_lab term."""
                gf_sb = gf_tiles[b]
                sig = []
                for k in range(2):
                    sg = spool.tile([128, NPRED], bf16, tag=f"sig{k}",
                                    name=f"sig{b}_{k}")
                    nc.scalar.activation(sg[:], bgb_sb[:], AF.Sigmoid,
                                         bias=gf_sb[:, k:k + 1])
                    sig.append(sg)

                mt_sb = dpool.tile([128, 4 * 128], bf16, tag="mt",
                                   name=f"mt{b}")
                pgt_t = pgtp.tile([NPRED, 128], f32, tag="pgt", name=f"pgt{b}")
                parts = []
                for k in range(2):
                    prg_t = prgp.tile([EPB, NPRED], f32, tag="prg",
                                      name=f"prg{b}_{k}")
                    nc.tensor.matmul(
                        prg_t[:],
                        srct_sb[:, (b * 2 + k) * EPB:(b * 2 + k + 1) * EPB],
                        sig[k][:], start=True, stop=True)
                    pg = spool.tile([EPB, NPRED], bf16, tag="pg",
                                    name=f"pg{b}_{k}")
                    nc.vector.tensor_mul(
                        pg[:], prg_t[:],
                        p1h_sb[:, b * NPRED:(b + 1) * NPRED])
                    nc.tensor.matmul(
                        pgt_t[:], pg[:],
                        tgto_sb[:, (b * 2 + k) * 128:(b * 2 + k + 1) * 128],
                        start=(k == 0), stop=False)
                    gcol = spool.tile([EPB, 1], f32, tag="gcol",
                                      name=f"gcol{b}_{k}")
                    nc.vector.tensor_reduce(gcol[:], pg[:],
                                            axis=mybir.AxisListType.X,
                                            op=ALU.add)
                    sides = [("L", plan["pb_lo"][(b, k)],
                              srcoL_sb[:, (b * 2 + k) * 128:(b * 2 + k + 1) * 128])]
                    if plan["pb_hi"][(b, k)] > plan["pb_lo"][(b, k)]:
                        ci = plan["cross_idx"][(b, k)]
                        sides.append(("H", plan["pb_hi"][(b, k)],
                                      srcoH_sb[:, ci * 128:(ci + 1) * 128]))
                    for si, (nmside, pb, srco_ap) in enumerate(sides):
                        slot = k * 2 + si
                        srcg = spool.tile([EPB, 128], bf16, tag="srcg",
                                          name=f"srcg{b}_{k}{nmside}")
                        nc.vector.tensor_scalar_mul(srcg[:], srco_ap, gcol[:])
                        pmt_t = pmtp.tile([128, 128], f32, tag="pmt",
                                          name=f"pmt{b}_{k}{nmside}")
                        nc.tensor.matmul(
                            pmt_t[:], srcg[:],
                            tgto_sb[:, (b * 2 + k) * 128:(b * 2 + k + 1) * 128],
                            start=True, stop=True)
                        nc.vector.tensor_copy(
                            out=mt_sb[:, slot * 128:(slot + 1) * 128],
                            in_=pmt_t[:])
                        parts.append((k, slot, pb))
                # self-loop: G row 0 += g2
                pg2 = spool.tile([128, NPRED], bf16, tag="pg2", name=f"pg2_{b}")
                nc.vector.tensor_scalar_mul(pg2[:], p1hs_sb[:], g2_tiles[b][:])
                nc.tensor.matmul(pgt_t[:], pg2[:], ident_sb[:],
                                 start=False, stop=True)
                gt_sb = dpool.tile([NPRED, 128], bf16, tag="gt", name=f"gt{b}")
                nc.vector.tensor_copy(out=gt_sb[:], in_=pgt_t[:])
                mtgt[b] = (mt_sb, gt_sb, parts)

            def stage3(b):
                mt_sb, gt_sb, parts = mtgt[b]
                pout_t = poutp.tile([128, CW], f32, tag="pout", name=f"po{b}")
                for i, (k, slot, pb) in enumerate(parts):
                    nc.tensor.matmul(pout_t[:],
                                     mt_sb[:, slot * 128:(slot + 1) * 128],
                                     hp_sb[(k, pb)][:],
                                     start=(i == 0), stop=False)
                nc.tensor.matmul(pout_t[:], gt_sb[:], blab_sb[:],
                                 start=False, stop=True)
                nc.vector.tensor_add(pout_t[:], pout_t[:], h2_sb[b][:])
                out_sb = opool.tile([128, CW], bf16, tag="out", name=f"ob{b}")
                nc.scalar.activation(out_sb[:], pout_t[:], AF.Relu)
                nc.sync.dma_start(out=out[b], in_=out_sb[:])
                del h2_sb[b], gf_tiles[b], g2_tiles[b], mtgt[b]
                del rft_tiles[b]
                for k in (0, 1):
                    for pb in {plan["pb_lo"][(b, k)], plan["pb_hi"][(b, k)]}:
                        if plan["last_user"][(k, pb)] == b:
                            del hp_sb[(k, pb)]

            # --- schedule: "heavy" passes = 16 k2gf + P0+P1 packed, with a
            # DEPTH-block k2 head start to absorb the W0/W1 DMA; packed
            # passes then alternate with the remaining k2 blocks.  builds
            # trail their k2gf block by one heavy pass; stage3(b) is emitted
            # as soon as its packed dependencies have been emitted. ---
            heavy = [("k2", b) for b in range(DEPTH)]
            pq = []
            for i in range(max(P0, P1)):
                if i < P0:
                    pq.append((0, i))
                if i < P1:
                    pq.append((1, i))
            bq = list(range(DEPTH, NBLK))
            while pq or bq:
                if pq:
                    heavy.append(("pp", pq.pop(0)))
                if bq:
                    heavy.append(("k2", bq.pop(0)))

            emitted_pb = {0: 0, 1: 0}
            built = set()
            done_upto = [0]  # next stage3 block

            def try_stage3():
                while (done_upto[0] < NBLK and done_upto[0] in built
                       and all(plan["pb_hi"][(done_upto[0], k)] < emitted_pb[k]
                               for k in (0, 1))):
                    stage3(done_upto[0])
                    done_upto[0] += 1

            pending_build = []
            prefetch_rft = DEPTH  # rft blocks already requested
            prefetch_pp = 2      # packed tiles already requested (1 per k)

            for hi, (kind, arg) in enumerate(heavy):
                if kind == "k2":
                    k2gf_pass(arg)
                    pending_build.append(arg)
                else:
                    k, pb = arg
                    ppass(k, pb)
                    emitted_pb[k] = pb + 1
                # prefetch DMA for upcoming heavy passes (~3 ahead)
                for j in range(hi + 1, min(hi + 4, len(heavy))):
                    kj, aj = heavy[j]
                    if kj == "k2" and aj >= prefetch_rft:
                        for bb in range(prefetch_rft, aj + 1):
                            _load_rft(bb)
                        prefetch_rft = aj + 1
                    if kj == "pp" and aj not in prft_tiles and aj[1] >= (
                            emitted_pb[aj[0]]):
                        if aj not in prft_tiles:
                            _load_prft(*aj)
                # builds trail by one heavy pass
                while len(pending_build) > 1:
                    bb = pending_build.pop(0)
                    build(bb)
                    built.add(bb)
                try_stage3()
            while pending_build:
                bb = pending_build.pop(0)
                build(bb)
                built.add(bb)
            try_stage3()
            assert done_upto[0] == NBLK, f"only {done_upto[0]} blocks done"

    nc.compile()
    return nc


def _host_prep(inputs, plan):
    rf = np.asarray(inputs["region_feats"], dtype=np.float32)
    W = np.asarray(inputs["W_conv"], dtype=np.float32)
    Wg = np.asarray(inputs["W_g"], dtype=np.float32)
    blab = np.asarray(inputs["b_lab"], dtype=np.float32)
    bglab = np.asarray(inputs["b_glab"], dtype=np.float32)
    rels = np.asarray(inputs["rels"])
    preds = np.asarray(inputs["pred_classes"])

    rels_r = rels.reshape(N_IMG, RPI, 3)[:, :NUM_REL].reshape(-1, 3)
    preds_r = preds.reshape(N_IMG, RPI)[:, :NUM_REL].reshape(-1)

    # RF^T tiles: rft_h[b, p, d*128+j] = RF[b*128+j, d*128+p]
    rft_h = np.ascontiguousarray(
        rf.T.reshape(32, 128, NBLK, 128).transpose(2, 1, 0, 3), dtype=BF
    ).reshape(NBLK, 128, 32 * 128)

    # packed RF^T tiles per (k, pb)
    P0, P1 = plan["P"][0], plan["P"][1]
    prft_h = np.zeros((P0 + P1, 128, 32 * 128), BF)
    prft8_h = np.zeros((P0, 128, 32 * 128), F8) if FP8K0 else None
    for k in (0, 1):
        u = plan["u"][k]
        for pb in range(plan["P"][k]):
            regs = u[pb * 128:(pb + 1) * 128]
            m = len(regs)
            # [m, D] -> [D, m] -> [32, 128, m] (d, p, j) -> (p, d, j)
            t = rf[regs].T.reshape(32, 128, m).transpose(1, 0, 2)
            i = pb if k == 0 else P0 + pb
            prft_h[i].reshape(128, 32, 128)[:, :, :m] = t.astype(BF)
            if FP8K0 and k == 0:
                prft8_h[pb].reshape(128, 32, 128)[:, :, :m] = _to_f8(t * S_R)

    # W slices per core, k-order (2, 0, 1):
    # w_h[p, ((ki*32+d)*CW)+j] = W[d*128+p, korder[ki]*D + c*CW + j]
    Wr = W.reshape(32, 128, 3, NCORES, CW)[:, :, (2, 0, 1)]
    w_cores = [
        np.ascontiguousarray(Wr[:, :, :, c, :].transpose(1, 2, 0, 3),
                             dtype=BF).reshape(128, 3 * 32 * CW)
        for c in range(NCORES)
    ]
    if FP8K0:
        # k=0 W slice (index 1 in (2,0,1) order), chunk-major, fp8-scaled
        w08_cores = [
            _to_f8(Wr[:, :, 1, c, :].transpose(1, 0, 2).reshape(128, 32 * CW)
                   * S_W)
            for c in range(NCORES)
        ]
    wg_h = np.ascontiguousarray(
        Wg.reshape(32, 128, 3).transpose(1, 0, 2), dtype=BF
    ).reshape(128, 32 * 3)
    blab_cores = [
        np.ascontiguousarray(blab[:, c * CW:(c + 1) * CW], dtype=BF)
        for c in range(NCORES)
    ]
    bgb_h = np.ascontiguousarray(
        np.repeat(bglab.reshape(1, NPRED), 128, axis=0), dtype=BF)

    srct_h = np.zeros((128, NBLK * 2 * EPB), np.float32)
    srcoL_h = np.zeros((EPB, NBLK * 2 * 128), np.float32)
    srcoH_h = np.zeros((EPB, plan["n_cross"] * 128), np.float32)
    tgto_h = np.zeros((EPB, NBLK * 2 * 128), np.float32)
    p1h_h = np.zeros((EPB, NBLK * NPRED), np.float32)
    e = np.arange(EPB)
    for b in range(NBLK):
        eb = rels_r[b * EPB:(b + 1) * EPB]
        pb_ = preds_r[b * EPB:(b + 1) * EPB]
        s = eb[:, 1] - b * 128
        o = eb[:, 2] - b * 128
        # k=0: obj -> subj (src=o, tgt=s); k=1: subj -> obj (src=s, tgt=o)
        for k, (src_loc, tgt_loc) in enumerate(((o, s), (s, o))):
            # fp8 path: descale factor folded into the packed-src one-hots
            oneval = 1.0 / (S_R * S_W) if (FP8K0 and k == 0) else 1.0
            srct_h[src_loc, (b * 2 + k) * EPB + e] = 1.0
            tgto_h[e, (b * 2 + k) * 128 + tgt_loc] = 1.0
            pk = plan["pos"][k][src_loc + b * 128]
            lo = plan["pb_lo"][(b, k)]
            hi = plan["pb_hi"][(b, k)]
            mlo = (pk // 128) == lo
            srcoL_h[e[mlo], (b * 2 + k) * 128 + (pk[mlo] - lo * 128)] = oneval
            if hi > lo:
                ci = plan["cross_idx"][(b, k)]
                mhi = ~mlo
                srcoH_h[e[mhi], ci * 128 + (pk[mhi] - hi * 128)] = oneval
        p1h_h[e, b * NPRED + pb_] = 1.0
    p1hs_h = np.zeros((128, NPRED), np.float32)
    p1hs_h[:, 0] = 1.0

    shared = {
        "rft": rft_h,
        "prft": prft_h,
        "wg": wg_h,
        "bgb": bgb_h,
        "srct": srct_h.astype(BF),
        "srcoL": srcoL_h.astype(BF),
        "srcoH": srcoH_h.astype(BF),
        "tgto": tgto_h.astype(BF),
        "p1h": p1h_h.astype(BF),
        "p1hs": p1hs_h.astype(BF),
        "ident": np.eye(128, dtype=np.float32).astype(BF),
    }
    if FP8K0:
        shared["prft8"] = prft8_h
    in_maps = []
    for c in range(NCORES):
        m = dict(shared)
        m["w"] = w_cores[c]
        m["blab"] = blab_cores[c]
        if FP8K0:
            m["w08"] = w08_cores[c]
        in_maps.append(m)
    return in_maps


def _rels_are_blocked(rels):
    """Check each image's relations reference only that image's regions."""
    rels = np.asarray(rels)
    if rels.shape != (N_IMG * RPI, 3):
        return False
    rels_r = rels.reshape(N_IMG, RPI, 3)[:, :NUM_REL]
    img = np.arange(N_IMG)[:, None]
    lo, hi = img * REG, (img + 1) * REG
    so = rels_r[:, :, 1:3]
    return bool(np.all((so >= lo[:, :, None]) & (so < hi[:, :, None])))


def _numpy_fallback(inputs):
    """Reference-equivalent host computation (only used if the per-image
    relation structure assumption is violated)."""
    rf = np.asarray(inputs["region_feats"], dtype=np.float32)
    W = np.asarray(inputs["W_conv"], dtype=np.float32)
    Wg = np.asarray(inputs["W_g"], dtype=np.float32)
    blab = np.asarray(inputs["b_lab"], dtype=np.float32)
    bglab = np.asarray(inputs["b_glab"], dtype=np.float32)
    rels = np.asarray(inputs["rels"])
    preds = np.asarray(inputs["pred_classes"])
    rels_r = rels.reshape(N_IMG, RPI, 3)[:, :NUM_REL].reshape(-1, 3)
    preds_r = preds.reshape(N_IMG, RPI)[:, :NUM_REL].reshape(-1)
    nf = (rf @ W).reshape(-1, D)
    gfe = (rf @ Wg).reshape(-1)
    s, o = rels_r[:, 1], rels_r[:, 2]
    self_ids = np.arange(N)
    idx = np.concatenate([o * 3 + 0, s * 3 + 1, self_ids * 3 + 2])
    pr = np.concatenate([preds_r, preds_r, np.zeros(N, preds_r.dtype)])
    tgt = np.concatenate([s, o, self_ids])
    gate = 1.0 / (1.0 + np.exp(-(gfe[idx] + bglab[pr, 0])))
    msg = gate[:, None] * (nf[idx] + blab[pr])
    upd = np.zeros((N, D), np.float32)
    np.add.at(upd, tgt, msg)
    return np.maximum(upd, 0.0)


def _emulate(inputs, plan, in_maps):
    """Numpy emulation of the device program from the STAGED tensors —
    validates host prep + on-device algebra (f32, no bf16 rounding)."""
    def f32(x):
        return np.asarray(x, dtype=np.float32)

    out = np.empty((N, D), np.float32)
    for c in range(NCORES):
        m = {k: f32(v) for k, v in in_maps[c].items()}
        w_k = {kk: m["w"][:, ki * 32 * CW:(ki + 1) * 32 * CW]
               for ki, kk in enumerate((2, 0, 1))}

        def lhsT_to_rows(tile):  # [128, 32*128] -> [128 rows, 4096]
            return tile.reshape(128, 32, 128).transpose(2, 1, 0).reshape(128, D)

        def wmat(kk):  # [128, 32*CW] -> [4096, CW]
            return w_k[kk].reshape(128, 32, CW).transpose(1, 0, 2).reshape(D, CW)

        hp = {}
        for k in (0, 1):
            for pb in range(plan["P"][k]):
                i = pb if k == 0 else plan["P"][0] + pb
                hp[(k, pb)] = lhsT_to_rows(m["prft"][i]) @ wmat(k)
        for b in range(NBLK):
            rows = lhsT_to_rows(m["rft"][b])
            h2 = rows @ wmat(2)
            gf = rows @ (m["wg"].reshape(128, 32, 3).transpose(1, 0, 2)
                         .reshape(D, 3))
            g2 = 1 / (1 + np.exp(-(m["bgb"][:, 0] + gf[:, 2])))
            h2s = g2[:, None] * h2
            pgt = np.zeros((NPRED, 128), np.float32)
            pout = np.zeros((128, CW), np.float32)
            for k in (0, 1):
                sig = 1 / (1 + np.exp(-(m["bgb"] + gf[:, k:k + 1])))
                sl = slice((b * 2 + k) * EPB, (b * 2 + k + 1) * EPB)
                sl128 = slice((b * 2 + k) * 128, (b * 2 + k + 1) * 128)
                prg = m["srct"][:, sl].T @ sig
                pg = prg * m["p1h"][:, b * NPRED:(b + 1) * NPRED]
                pgt += pg.T @ m["tgto"][:, sl128]
                gcol = pg.sum(1)
                sides = [(plan["pb_lo"][(b, k)], m["srcoL"][:, sl128])]
                if plan["pb_hi"][(b, k)] > plan["pb_lo"][(b, k)]:
                    ci = plan["cross_idx"][(b, k)]
                    sides.append((plan["pb_hi"][(b, k)],
                                  m["srcoH"][:, ci * 128:(ci + 1) * 128]))
                for pb, srco in sides:
                    srcg = srco * gcol[:, None]
                    pmt = srcg.T @ m["tgto"][:, sl128]
                    pout += pmt.T @ hp[(k, pb)]
            pg2 = m["p1hs"] * g2[:, None]
            pgt += pg2.T
            pout += pgt.T @ m["blab"]
            out[b * 128:(b + 1) * 128, c * CW:(c + 1) * CW] = np.maximum(
                pout + h2s, 0.0)
    return out


def _run(inputs, trace=False):
    from concourse.bass_utils import run_bass_kernel_spmd

    rels = np.asarray(inputs["rels"])
    plan = _plan_packing(rels)
    key = rels.tobytes()
    if key not in _prog_cache:
        _prog_cache.clear()
        _prog_cache[key] = _build_program(plan)
    nc = _prog_cache[key]
    in_maps = _host_prep(inputs, plan)
    try:
        res = run_bass_kernel_spmd(nc, in_maps, core_ids=list(range(NCORES)),
                                   trace=trace)
    except Exception:
        # transient device errors (e.g. NRT_EXEC_UNIT_UNRECOVERABLE) have
        # been observed to clear on retry
        import time
        time.sleep(5)
        res = run_bass_kernel_spmd(nc, in_maps, core_ids=list(range(NCORES)),
                                   trace=trace)
    out = np.empty((N, D), np.float32)
    for c in range(NCORES):
        out[:, c * CW:(c + 1) * CW] = np.asarray(
            res.results[c]["out"], dtype=np.float32).reshape(N, CW)
    return out, res


def kernel(**inputs):
    if not _rels_are_blocked(inputs["rels"]):
        return _numpy_fallback(inputs)
    out, _ = _run(inputs, trace=False)
    return out


# revision 3
# speedup vs baseline: 1.0048x; 1.0048x over previous
"""GCN message-passing kernel for Trainium2 (8 NeuronCores, SPMD) — v2.

Math (matches the reference):
    gf   = RF @ W_g                          (2048, 3)   gate features
    H_k  = RF @ W_k                          (2048, 4096) per edge type k in {0,1,2}
    gate(e) = sigmoid(gf[src_e, k_e] + b_glab[p_e])
    upd[t]  = sum_{e->t} gate(e) * (H_{k_e}[src_e] + b_lab[p_e])
    out  = relu(upd)

v2 key idea (on top of the v1 block-diagonal gate-matrix formulation):
only ~47% of regions ever appear as a message SOURCE for edge types 0/1
(unique objs/subjs per image ~15 of 32).  H_0/H_1 rows for unused
sources are dead work.  We therefore compute H_0/H_1 only for the
PACKED list of unique sources: the used RF rows are compacted (host
side) into ~8 pseudo-blocks of 128 per edge type instead of 16 full
blocks, cutting the dominant PE matmul stream from 48 to ~32
block-passes (~2/3 of v1's tensor-engine work).  The self-loop type
k=2 still needs every region, and the per-region gate features gf ride
along the k=2 pass as interleaved 3-column matmuls.

Aggregation stays on device: per target block, block-local gate
matrices in PACKED source coordinates are built from one-hot index
matrices (host prepares 0/1 matrices from rels; gates themselves are
computed on Trainium).  A target block's sources span at most 2
pseudo-blocks (sources are packed in region order), so stage3 needs at
most 2 matmuls per edge type.

Sharding: unchanged from v1 — output D dim split 8 ways; each core
computes all 2048 rows x its 512 columns.  No collectives.

The pseudo-block structure depends on `rels`, so the program is
compiled per relation pattern (cached by rels bytes).
"""

import numpy as np
import ml_dtypes

# problem constants (hardcoded per contract)
N_IMG = 64
REG = 32
RPI = 32
NUM_REL = 20
D = 4096
NPRED = 81
N = N_IMG * REG          # 2048
NCORES = 8
CW = D // NCORES         # 512 output cols per core
NBLK = N // 128          # 16 row blocks
IPB = 128 // REG         # 4 images per block
EPB = IPB * NUM_REL      # 80 edges per block per edge type

BF = ml_dtypes.bfloat16
F8 = ml_dtypes.float8_e4m3fn

import os
FP8K0 = os.environ.get("FP8K0", "0") == "1"  # probe: k0 packed passes in fp8
S_R = 32.0     # fp8 scale for RF
S_W = 2048.0   # fp8 scale for W

_prog_cache = {}


def _to_f8(x):
    return np.clip(x, -440.0, 440.0).astype(F8)


def _plan_packing(rels):
    """Host-side packing plan from the relation list.

    Returns a dict with, per edge type k in (0, 1):
      u[k]        sorted unique global source region ids
      P[k]        number of 128-wide pseudo-blocks
      pos[k]      region id -> packed position (array of len N, -1 unused)
    and per (block b, k):
      pb_lo/pb_hi pseudo-blocks containing this block's sources
      cross       whether pb_hi > pb_lo
    plus cross-index assignment for the compact hi-side one-hot tensor.
    """
    rels_r = np.asarray(rels).reshape(N_IMG, RPI, 3)[:, :NUM_REL].reshape(-1, 3)
    s, o = rels_r[:, 1], rels_r[:, 2]
    srcs = {0: o, 1: s}
    plan = {"u": {}, "P": {}, "pos": {}, "pb_lo": {}, "pb_hi": {},
            "cross_idx": {}, "n_cross": 0, "last_user": {}}
    for k in (0, 1):
        u = np.unique(srcs[k])
        pos = np.full(N, -1, np.int64)
        pos[u] = np.arange(len(u))
        plan["u"][k] = u
        plan["P"][k] = -(-len(u) // 128)
        plan["pos"][k] = pos
    n_cross = 0
    for b in range(NBLK):
        for k in (0, 1):
            pk = plan["pos"][k][srcs[k][b * EPB:(b + 1) * EPB]]
            assert np.all(pk >= 0)
            pbs = pk // 128
            lo, hi = int(pbs.min()), int(pbs.max())
            assert hi - lo <= 1, f"block {b} k {k} spans {hi - lo + 1} pbs"
            plan["pb_lo"][(b, k)] = lo
            plan["pb_hi"][(b, k)] = hi
            if hi > lo:
                plan["cross_idx"][(b, k)] = n_cross
                n_cross += 1
            for pb in (lo, hi):
                key = (k, pb)
                plan["last_user"][key] = max(plan["last_user"].get(key, -1), b)
    plan["n_cross"] = max(n_cross, 1)
    return plan


def _plan_key(plan):
    items = [tuple(sorted(plan["u"][k].tolist())) for k in (0, 1)]
    items.append(tuple(sorted(plan["pb_lo"].items())))
    items.append(tuple(sorted(plan["pb_hi"].items())))
    return hash(tuple(map(tuple, map(str, items))))


def _build_program(plan):
    import concourse.bass as bass
    import concourse.tile as tile
    from concourse import bacc, mybir

    bf16 = mybir.dt.bfloat16
    f32 = mybir.dt.float32
    AF = mybir.ActivationFunctionType
    ALU = mybir.AluOpType

    P0, P1 = plan["P"][0], plan["P"][1]
    NCROSS = plan["n_cross"]

    nc = bacc.Bacc("TRN2", target_bir_lowering=False, debug=False,
                   num_devices=NCORES)

    f8e4 = mybir.dt.float8e4

    rft = nc.dram_tensor("rft", [NBLK, 128, 32 * 128], bf16, kind="ExternalInput").ap()
    prft = nc.dram_tensor("prft", [P0 + P1, 128, 32 * 128], bf16, kind="ExternalInput").ap()
    if FP8K0:
        prft8 = nc.dram_tensor("prft8", [P0, 128, 32 * 128], f8e4,
                               kind="ExternalInput").ap()
        w08 = nc.dram_tensor("w08", [128, 32 * CW], f8e4,
                             kind="ExternalInput").ap()
    # w layout: k-order (2, 0, 1), i.e. cols [k2 | k0 | k1], chunk-major inside
    w = nc.dram_tensor("w", [128, 3 * 32 * CW], bf16, kind="ExternalInput").ap()
    wg = nc.dram_tensor("wg", [128, 32 * 3], bf16, kind="ExternalInput").ap()
    blab = nc.dram_tensor("blab", [NPRED, CW], bf16, kind="ExternalInput").ap()
    bgb = nc.dram_tensor("bgb", [128, NPRED], bf16, kind="ExternalInput").ap()
    srct = nc.dram_tensor("srct", [128, NBLK * 2 * EPB], bf16, kind="ExternalInput").ap()
    srcoL = nc.dram_tensor("srcoL", [EPB, NBLK * 2 * 128], bf16, kind="ExternalInput").ap()
    srcoH = nc.dram_tensor("srcoH", [EPB, NCROSS * 128], bf16, kind="ExternalInput").ap()
    tgto = nc.dram_tensor("tgto", [EPB, NBLK * 2 * 128], bf16, kind="ExternalInput").ap()
    p1h = nc.dram_tensor("p1h", [EPB, NBLK * NPRED], bf16, kind="ExternalInput").ap()
    p1hs = nc.dram_tensor("p1hs", [128, NPRED], bf16, kind="ExternalInput").ap()
    ident = nc.dram_tensor("ident", [128, 128], bf16, kind="ExternalInput").ap()
    out = nc.dram_tensor("out", [NBLK, 128, CW], bf16, kind="ExternalOutput").ap()

    DEPTH = 6  # k2 blocks run before the first packed pass (absorbs W DMA);
    # only 4 rft tiles are loaded upfront — blocks 4/5 come via prefetch so
    # no startup DMA ever WAR-blocks on pool buffer recycling
    with tile.TileContext(nc) as tc:
        with (
            tc.tile_pool(name="consts", bufs=1) as cpool,
            tc.tile_pool(name="rft", bufs=4) as rpool,
            tc.tile_pool(name="prft", bufs=2) as ppool,
            tc.tile_pool(name="hp", bufs=5) as hppool,
            tc.tile_pool(name="deep", bufs=9) as dpool,
            tc.tile_pool(name="small", bufs=2) as spool,
            tc.tile_pool(name="osb", bufs=2) as opool,
            tc.tile_pool(name="ph", bufs=2, space="PSUM") as php,
            tc.tile_pool(name="pgf", bufs=1, space="PSUM") as pgfp,
            tc.tile_pool(name="prg", bufs=1, space="PSUM") as prgp,
            tc.tile_pool(name="pgt", bufs=1, space="PSUM") as pgtp,
            tc.tile_pool(name="pmt", bufs=2, space="PSUM") as pmtp,
            tc.tile_pool(name="pout", bufs=1, space="PSUM") as poutp,
        ):
            # --- input DMAs, ordered by when the PE needs each tensor ---
            wg_sb = cpool.tile([128, 32 * 3], bf16, tag="wg")
            nc.sync.dma_start(out=wg_sb[:], in_=wg[:])
            WCH = 4 * CW  # w chunk: 4 d-tiles
            w_ch = {k: [cpool.tile([128, WCH], bf16, tag=f"w{k}c{i}",
                                   name=f"w{k}c{i}")
                        for i in range(8)]
                    for k in ((2, 1) if FP8K0 else (2, 0, 1))}

            def _rft_half(b, h):
                t = rpool.tile([128, 16 * 128], bf16, tag=f"rft{h}",
                               name=f"rft{h}_{b}")
                nc.sync.dma_start(out=t[:],
                                  in_=rft[b, :, h * 16 * 128:(h + 1) * 16 * 128])
                return t

            def _load_rft(b):
                rft_tiles[b] = [_rft_half(b, 0), _rft_half(b, 1)]

            def _prft_half(i, h):
                t = ppool.tile([128, 16 * 128], bf16, tag=f"prft{h}",
                               name=f"prft{h}_{i}")
                nc.sync.dma_start(out=t[:],
                                  in_=prft[i, :, h * 16 * 128:(h + 1) * 16 * 128])
                return t

            def _prft8_half(i, h):
                t = ppool.tile([128, 16, 128], f8e4, tag=f"p8_{h}",
                               name=f"p8_{h}_{i}")
                nc.sync.dma_start(out=t[:],
                                  in_=prft8[i, :, h * 16 * 128:(h + 1) * 16 * 128])
                return t

            def _load_prft(k, pb):
                if FP8K0 and k == 0:
                    prft_tiles[(k, pb)] = [_prft8_half(pb, 0), _prft8_half(pb, 1)]
                else:
                    i = pb if k == 0 else P0 + pb
                    prft_tiles[(k, pb)] = [_prft_half(i, 0), _prft_half(i, 1)]

            if FP8K0:
                w08_sb = cpool.tile([128, 32, CW], f8e4, tag="w08",
                                    name="w08")

            def _load_w(k):
                if FP8K0 and k == 0:
                    nc.sync.dma_start(out=w08_sb[:], in_=w08[:])
                    return
                # dram w layout is k-order (2, 0, 1), chunk-major
                base = {2: 0, 0: 1, 1: 2}[k] * 32 * CW
                for i in range(8):
                    nc.sync.dma_start(out=w_ch[k][i][:],
                                      in_=w[:, base + i * WCH:
                                            base + (i + 1) * WCH])

            rft_tiles, prft_tiles = {}, {}
            nc.sync.dma_start(out=w_ch[2][0][:], in_=w[:, 0:WCH])
            _load_rft(0)
            for i in range(1, 8):
                nc.sync.dma_start(out=w_ch[2][i][:],
                                  in_=w[:, i * WCH:(i + 1) * WCH])
            # small consts needed by build(0) early
            blab_sb = cpool.tile([NPRED, CW], bf16, tag="blab")
            nc.sync.dma_start(out=blab_sb[:], in_=blab[:])
            bgb_sb = cpool.tile([128, NPRED], bf16, tag="bgb")
            nc.sync.dma_start(out=bgb_sb[:], in_=bgb[:])
            p1hs_sb = cpool.tile([128, NPRED], bf16, tag="p1hs")
            nc.sync.dma_start(out=p1hs_sb[:], in_=p1hs[:])
            ident_sb = cpool.tile([128, 128], bf16, tag="ident")
            nc.sync.dma_start(out=ident_sb[:], in_=ident[:])
            _load_rft(1)
            srct_sb = cpool.tile([128, NBLK * 2 * EPB], bf16, tag="srct")
            nc.sync.dma_start(out=srct_sb[:], in_=srct[:])
            p1h_sb = cpool.tile([EPB, NBLK * NPRED], bf16, tag="p1h")
            nc.sync.dma_start(out=p1h_sb[:], in_=p1h[:])
            _load_rft(2)
            srcoL_sb = cpool.tile([EPB, NBLK * 2 * 128], bf16, tag="srcoL")
            nc.sync.dma_start(out=srcoL_sb[:], in_=srcoL[:])
            srcoH_sb = cpool.tile([EPB, NCROSS * 128], bf16, tag="srcoH")
            nc.sync.dma_start(out=srcoH_sb[:], in_=srcoH[:])
            tgto_sb = cpool.tile([EPB, NBLK * 2 * 128], bf16, tag="tgto")
            nc.sync.dma_start(out=tgto_sb[:], in_=tgto[:])
            _load_rft(3)
            _load_prft(0, 0)
            _load_w(0)
            # prft(1,0) and W1 are issued at emission time (heavy pass 2),
            # AFTER the rft4/rft5 prefetches — earliest-deadline order in the
            # in-order DMA queues (they are not needed until heavy pass 8)

            h2_sb, hp_sb, gf_tiles, g2_tiles, mtgt = {}, {}, {}, {}, {}

            def rft_lhsT(b, d):
                return rft_tiles[b][d // 16][:, (d % 16) * 128:(d % 16 + 1) * 128]

            def prft_lhsT(k, pb, d):
                return prft_tiles[(k, pb)][d // 16][:, (d % 16) * 128:(d % 16 + 1) * 128]

            from concourse.tile_rust import add_dep_helper

            def k2gf_pass(b):
                """H_2(b) = RF_b @ W_2 with gf(b) interleaved (gf matmuls
                reuse the H matmul's stationary operand via ldweights=False;
                the local ordering chain keeps each gf adjacent to its H
                partner).  The self-loop gate is folded into the PSUM->SBUF
                copy: h2s = diag(g2) @ H_2."""
                ph_t = php.tile([128, CW], f32, tag="ph", name=f"ph{b}_2")
                pgf_t = pgfp.tile([128, 3], f32, tag="pgf", name=f"pgf{b}")
                prev = None
                for d in range(32):
                    lhsT = rft_lhsT(b, d)
                    nc.tensor.matmul(ph_t[:], lhsT,
                                     w_ch[2][d // 4][:, (d % 4) * CW:(d % 4 + 1) * CW],
                                     start=(d == 0), stop=(d == 31))
                    h_inst = nc.main_func.blocks[-1].instructions[-1]
                    assert h_inst.opcode == "Matmult"
                    if prev is not None:
                        add_dep_helper(h_inst, prev, sync=False,
                                       reason="k2-chain")
                    nc.tensor.matmul(pgf_t[:], lhsT,
                                     wg_sb[:, d * 3:(d + 1) * 3],
                                     start=(d == 0), stop=(d == 31))
                    gf_inst = nc.main_func.blocks[-1].instructions[-1]
                    assert gf_inst.opcode == "Matmult"
                    gf_inst.ldweights = False
                    add_dep_helper(gf_inst, h_inst, sync=False,
                                   reason="k2-pair")
                    prev = gf_inst
                gf_sb = dpool.tile([128, 3], f32, tag="gf", name=f"gf{b}")
                nc.vector.tensor_copy(out=gf_sb[:], in_=pgf_t[:])
                gf_tiles[b] = gf_sb
                g2 = dpool.tile([128, 1], f32, tag="g2", name=f"g2_{b}")
                nc.scalar.activation(g2[:], bgb_sb[:, 0:1], AF.Sigmoid,
                                     bias=gf_sb[:, 2:3])
                g2_tiles[b] = g2
                hk = dpool.tile([128, CW], bf16, tag="h2", name=f"h2_{b}")
                nc.vector.tensor_scalar_mul(hk[:], ph_t[:], g2[:])
                h2_sb[b] = hk

            def ppass(k, pb):
                """Packed H_k rows for pseudo-block pb."""
                ph_t = php.tile([128, CW], f32, tag="ph", name=f"php{k}_{pb}")
                if FP8K0 and k == 0:
                    # fp8 DoubleRow: chunk-pair (K=256) per matmul; lhsT free
                    # [2,128] and rhs free [2,512] carry an explicit pair dim
                    for dp in range(16):
                        half = prft_tiles[(k, pb)][dp // 8]
                        j = dp % 8
                        lhsT = half[:, 2 * j:2 * j + 2, :]
                        rhs = w08_sb[:, 2 * dp:2 * dp + 2, :]
                        nc.tensor.matmul(
                            ph_t[:], lhsT, rhs, start=(dp == 0),
                            stop=(dp == 15),
                            perf_mode=mybir.MatmulPerfMode.DoubleRow)
                else:
                    for d in range(32):
                        nc.tensor.matmul(ph_t[:], prft_lhsT(k, pb, d),
                                         w_ch[k][d // 4][:, (d % 4) * CW:(d % 4 + 1) * CW],
                                         start=(d == 0), stop=(d == 31))
                hk = hppool.tile([128, CW], bf16, tag=f"hp{k}",
                                 name=f"hp{k}_{pb}")
                nc.vector.tensor_copy(out=hk[:], in_=ph_t[:])
                hp_sb[(k, pb)] = hk
                del prft_tiles[(k, pb)]

            def build(b):
                """Gates -> packed-coordinate gate matrices (lhsT form,
                [packed_pos_in_pb, target]) and G^T for the b_lab term."""
                gf_sb = gf_tiles[b]
                sig = []
                for k in range(2):
                    sg = spool.tile([128, NPRED], bf16, tag=f"sig{k}",
                                    name=f"sig{b}_{k}")
                    nc.scalar.activation(sg[:], bgb_sb[:], AF.Sigmoid,
                                         bias=gf_sb[:, k:k + 1])
                    sig.append(sg)

                mt_sb = dpool.tile([128, 4 * 128], bf16, tag="mt",
                                   name=f"mt{b}")
                pgt_t = pgtp.tile([NPRED, 128], f32, tag="pgt", name=f"pgt{b}")
                parts = []
                for k in range(2):
                    prg_t = prgp.tile([EPB, NPRED], f32, tag="prg",
                                      name=f"prg{b}_{k}")
                    nc.tensor.matmul(
                        prg_t[:],
                        srct_sb[:, (b * 2 + k) * EPB:(b * 2 + k + 1) * EPB],
                        sig[k][:], start=True, stop=True)
                    pg = spool.tile([EPB, NPRED], bf16, tag="pg",
                                    name=f"pg{b}_{k}")
                    nc.vector.tensor_mul(
                        pg[:], prg_t[:],
                        p1h_sb[:, b * NPRED:(b + 1) * NPRED])
                    nc.tensor.matmul(
                        pgt_t[:], pg[:],
                        tgto_sb[:, (b * 2 + k) * 128:(b * 2 + k + 1) * 128],
                        start=(k == 0), stop=False)
                    gcol = spool.tile([EPB, 1], f32, tag="gcol",
                                      name=f"gcol{b}_{k}")
                    nc.vector.tensor_reduce(gcol[:], pg[:],
                                            axis=mybir.AxisListType.X,
                                            op=ALU.add)
                    sides = [("L", plan["pb_lo"][(b, k)],
                              srcoL_sb[:, (b * 2 + k) * 128:(b * 2 + k + 1) * 128])]
                    if plan["pb_hi"][(b, k)] > plan["pb_lo"][(b, k)]:
                        ci = plan["cross_idx"][(b, k)]
                        sides.append(("H", plan["pb_hi"][(b, k)],
                                      srcoH_sb[:, ci * 128:(ci + 1) * 128]))
                    for si, (nmside, pb, srco_ap) in enumerate(sides):
                        slot = k * 2 + si
                        srcg = spool.tile([EPB, 128], bf16, tag="srcg",
                                          name=f"srcg{b}_{k}{nmside}")
                        nc.vector.tensor_scalar_mul(srcg[:], srco_ap, gcol[:])
                        pmt_t = pmtp.tile([128, 128], f32, tag="pmt",
                                          name=f"pmt{b}_{k}{nmside}")
                        nc.tensor.matmul(
                            pmt_t[:], srcg[:],
                            tgto_sb[:, (b * 2 + k) * 128:(b * 2 + k + 1) * 128],
                            start=True, stop=True)
                        nc.vector.tensor_copy(
                            out=mt_sb[:, slot * 128:(slot + 1) * 128],
                            in_=pmt_t[:])
                        parts.append((k, slot, pb))
                # self-loop: G row 0 += g2
                pg2 = spool.tile([128, NPRED], bf16, tag="pg2", name=f"pg2_{b}")
                nc.vector.tensor_scalar_mul(pg2[:], p1hs_sb[:], g2_tiles[b][:])
                nc.tensor.matmul(pgt_t[:], pg2[:], ident_sb[:],
                                 start=False, stop=True)
                gt_sb = dpool.tile([NPRED, 128], bf16, tag="gt", name=f"gt{b}")
                nc.vector.tensor_copy(out=gt_sb[:], in_=pgt_t[:])
                mtgt[b] = (mt_sb, gt_sb, parts)

            def stage3(b):
                mt_sb, gt_sb, parts = mtgt[b]
                pout_t = poutp.tile([128, CW], f32, tag="pout", name=f"po{b}")
                for i, (k, slot, pb) in enumerate(parts):
                    nc.tensor.matmul(pout_t[:],
                                     mt_sb[:, slot * 128:(slot + 1) * 128],
                                     hp_sb[(k, pb)][:],
                                     start=(i == 0), stop=False)
                nc.tensor.matmul(pout_t[:], gt_sb[:], blab_sb[:],
                                 start=False, stop=True)
                nc.vector.tensor_add(pout_t[:], pout_t[:], h2_sb[b][:])
                out_sb = opool.tile([128, CW], bf16, tag="out", name=f"ob{b}")
                nc.scalar.activation(out_sb[:], pout_t[:], AF.Relu)
                nc.sync.dma_start(out=out[b], in_=out_sb[:])
                del h2_sb[b], gf_tiles[b], g2_tiles[b], mtgt[b]
                del rft_tiles[b]
                for k in (0, 1):
                    for pb in {plan["pb_lo"][(b, k)], plan["pb_hi"][(b, k)]}:
                        if plan["last_user"][(k, pb)] == b:
                            del hp_sb[(k, pb)]

            # --- schedule: "heavy" passes = 16 k2gf + P0+P1 packed, with a
            # DEPTH-block k2 head start to absorb the W0/W1 DMA; packed
            # passes then alternate with the remaining k2 blocks.  builds
            # trail their k2gf block by one heavy pass; stage3(b) is emitted
            # as soon as its packed dependencies have been emitted. ---
            heavy = [("k2", b) for b in range(DEPTH)]
            pq = []
            for i in range(max(P0, P1)):
                if i < P0:
                    pq.append((0, i))
                if i < P1:
                    pq.append((1, i))
            bq = list(range(DEPTH, NBLK))
            while pq or bq:
                if pq:
                    heavy.append(("pp", pq.pop(0)))
                if bq:
                    heavy.append(("k2", bq.pop(0)))

            emitted_pb = {0: 0, 1: 0}
            built = set()
            done_upto = [0]  # next stage3 block

            def try_stage3():
                while (done_upto[0] < NBLK and done_upto[0] in built
                       and all(plan["pb_hi"][(done_upto[0], k)] < emitted_pb[k]
                               for k in (0, 1))):
                    stage3(done_upto[0])
                    done_upto[0] += 1

            pending_build = []
            prefetch_rft = 4     # rft blocks already requested upfront
            prefetch_pp = 2      # packed tiles already requested (1 per k)

            for hi, (kind, arg) in enumerate(heavy):
                if kind == "k2":
                    k2gf_pass(arg)
                    pending_build.append(arg)
                else:
                    k, pb = arg
                    ppass(k, pb)
                    emitted_pb[k] = pb + 1
                # deadline-ordered late loads: rft4/5 recycle buffers of
                # blocks 0/1 (WAR resolves after their k2gf), then W1+prft10
                if hi == 0:
                    _load_rft(4)
                    prefetch_rft = 5
                elif hi == 1:
                    _load_rft(5)
                    prefetch_rft = 6
                elif hi == 2:
                    _load_prft(1, 0)
                    _load_w(1)
                # prefetch DMA for upcoming heavy passes (~3 ahead)
                for j in range(hi + 1, min(hi + 4, len(heavy))):
                    kj, aj = heavy[j]
                    if kj == "k2" and aj >= prefetch_rft:
                        for bb in range(prefetch_rft, aj + 1):
                            _load_rft(bb)
                        prefetch_rft = aj + 1
                    if kj == "pp" and aj not in prft_tiles and aj[1] >= (
                            emitted_pb[aj[0]]):
                        if aj not in prft_tiles:
                            _load_prft(*aj)
                # builds trail by one heavy pass
                while len(pending_build) > 1:
                    bb = pending_build.pop(0)
                    build(bb)
                    built.add(bb)
                try_stage3()
            while pending_build:
                bb = pending_build.pop(0)
                build(bb)
                built.add(bb)
            try_stage3()
            assert done_upto[0] == NBLK, f"only {done_upto[0]} blocks done"

    nc.compile()
    return nc


def _host_prep(inputs, plan):
    rf = np.asarray(inputs["region_feats"], dtype=np.float32)
    W = np.asarray(inputs["W_conv"], dtype=np.float32)
    Wg = np.asarray(inputs["W_g"], dtype=np.float32)
    blab = np.asarray(inputs["b_lab"], dtype=np.float32)
    bglab = np.asarray(inputs["b_glab"], dtype=np.float32)
    rels = np.asarray(inputs["rels"])
    preds = np.asarray(inputs["pred_classes"])

    rels_r = rels.reshape(N_IMG, RPI, 3)[:, :NUM_REL].reshape(-1, 3)
    preds_r = preds.reshape(N_IMG, RPI)[:, :NUM_REL].reshape(-1)

    # RF^T tiles: rft_h[b, p, d*128+j] = RF[b*128+j, d*128+p]
    rft_h = np.ascontiguousarray(
        rf.T.reshape(32, 128, NBLK, 128).transpose(2, 1, 0, 3), dtype=BF
    ).reshape(NBLK, 128, 32 * 128)

    # packed RF^T tiles per (k, pb)
    P0, P1 = plan["P"][0], plan["P"][1]
    prft_h = np.zeros((P0 + P1, 128, 32 * 128), BF)
    prft8_h = np.zeros((P0, 128, 32 * 128), F8) if FP8K0 else None
    for k in (0, 1):
        u = plan["u"][k]
        for pb in range(plan["P"][k]):
            regs = u[pb * 128:(pb + 1) * 128]
            m = len(regs)
            # [m, D] -> [D, m] -> [32, 128, m] (d, p, j) -> (p, d, j)
            t = rf[regs].T.reshape(32, 128, m).transpose(1, 0, 2)
            i = pb if k == 0 else P0 + pb
            prft_h[i].reshape(128, 32, 128)[:, :, :m] = t.astype(BF)
            if FP8K0 and k == 0:
                prft8_h[pb].reshape(128, 32, 128)[:, :, :m] = _to_f8(t * S_R)

    # W slices per core, k-order (2, 0, 1):
    # w_h[p, ((ki*32+d)*CW)+j] = W[d*128+p, korder[ki]*D + c*CW + j]
    Wr = W.reshape(32, 128, 3, NCORES, CW)[:, :, (2, 0, 1)]
    w_cores = [
        np.ascontiguousarray(Wr[:, :, :, c, :].transpose(1, 2, 0, 3),
                             dtype=BF).reshape(128, 3 * 32 * CW)
        for c in range(NCORES)
    ]
    if FP8K0:
        # k=0 W slice (index 1 in (2,0,1) order), chunk-major, fp8-scaled
        w08_cores = [
            _to_f8(Wr[:, :, 1, c, :].transpose(1, 0, 2).reshape(128, 32 * CW)
                   * S_W)
            for c in range(NCORES)
        ]
    wg_h = np.ascontiguousarray(
        Wg.reshape(32, 128, 3).transpose(1, 0, 2), dtype=BF
    ).reshape(128, 32 * 3)
    blab_cores = [
        np.ascontiguousarray(blab[:, c * CW:(c + 1) * CW], dtype=BF)
        for c in range(NCORES)
    ]
    bgb_h = np.ascontiguousarray(
        np.repeat(bglab.reshape(1, NPRED), 128, axis=0), dtype=BF)

    srct_h = np.zeros((128, NBLK * 2 * EPB), np.float32)
    srcoL_h = np.zeros((EPB, NBLK * 2 * 128), np.float32)
    srcoH_h = np.zeros((EPB, plan["n_cross"] * 128), np.float32)
    tgto_h = np.zeros((EPB, NBLK * 2 * 128), np.float32)
    p1h_h = np.zeros((EPB, NBLK * NPRED), np.float32)
    e = np.arange(EPB)
    for b in range(NBLK):
        eb = rels_r[b * EPB:(b + 1) * EPB]
        pb_ = preds_r[b * EPB:(b + 1) * EPB]
        s = eb[:, 1] - b * 128
        o = eb[:, 2] - b * 128
        # k=0: obj -> subj (src=o, tgt=s); k=1: subj -> obj (src=s, tgt=o)
        for k, (src_loc, tgt_loc) in enumerate(((o, s), (s, o))):
            # fp8 path: descale factor folded into the packed-src one-hots
            oneval = 1.0 / (S_R * S_W) if (FP8K0 and k == 0) else 1.0
            srct_h[src_loc, (b * 2 + k) * EPB + e] = 1.0
            tgto_h[e, (b * 2 + k) * 128 + tgt_loc] = 1.0
            pk = plan["pos"][k][src_loc + b * 128]
            lo = plan["pb_lo"][(b, k)]
            hi = plan["pb_hi"][(b, k)]
            mlo = (pk // 128) == lo
            srcoL_h[e[mlo], (b * 2 + k) * 128 + (pk[mlo] - lo * 128)] = oneval
            if hi > lo:
                ci = plan["cross_idx"][(b, k)]
                mhi = ~mlo
                srcoH_h[e[mhi], ci * 128 + (pk[mhi] - hi * 128)] = oneval
        p1h_h[e, b * NPRED + pb_] = 1.0
    p1hs_h = np.zeros((128, NPRED), np.float32)
    p1hs_h[:, 0] = 1.0

    shared = {
        "rft": rft_h,
        "prft": prft_h,
        "wg": wg_h,
        "bgb": bgb_h,
        "srct": srct_h.astype(BF),
        "srcoL": srcoL_h.astype(BF),
        "srcoH": srcoH_h.astype(BF),
        "tgto": tgto_h.astype(BF),
        "p1h": p1h_h.astype(BF),
        "p1hs": p1hs_h.astype(BF),
        "ident": np.eye(128, dtype=np.float32).astype(BF),
    }
    if FP8K0:
        shared["prft8"] = prft8_h
    in_maps = []
    for c in range(NCORES):
        m = dict(shared)
        m["w"] = w_cores[c]
        m["blab"] = blab_cores[c]
        if FP8K0:
            m["w08"] = w08_cores[c]
        in_maps.append(m)
    return in_maps


def _rels_are_blocked(rels):
    """Check each image's relations reference only that image's regions."""
    rels = np.asarray(rels)
    if rels.shape != (N_IMG * RPI, 3):
        return False
    rels_r = rels.reshape(N_IMG, RPI, 3)[:, :NUM_REL]
    img = np.arange(N_IMG)[:, None]
    lo, hi = img * REG, (img + 1) * REG
    so = rels_r[:, :, 1:3]
    return bool(np.all((so >= lo[:, :, None]) & (so < hi[:, :, None])))


def _numpy_fallback(inputs):
    """Reference-equivalent host computation (only used if the per-image
    relation structure assumption is violated)."""
    rf = np.asarray(inputs["region_feats"], dtype=np.float32)
    W = np.asarray(inputs["W_conv"], dtype=np.float32)
    Wg = np.asarray(inputs["W_g"], dtype=np.float32)
    blab = np.asarray(inputs["b_lab"], dtype=np.float32)
    bglab = np.asarray(inputs["b_glab"], dtype=np.float32)
    rels = np.asarray(inputs["rels"])
    preds = np.asarray(inputs["pred_classes"])
    rels_r = rels.reshape(N_IMG, RPI, 3)[:, :NUM_REL].reshape(-1, 3)
    preds_r = preds.reshape(N_IMG, RPI)[:, :NUM_REL].reshape(-1)
    nf = (rf @ W).reshape(-1, D)
    gfe = (rf @ Wg).reshape(-1)
    s, o = rels_r[:, 1], rels_r[:, 2]
    self_ids = np.arange(N)
    idx = np.concatenate([o * 3 + 0, s * 3 + 1, self_ids * 3 + 2])
    pr = np.concatenate([preds_r, preds_r, np.zeros(N, preds_r.dtype)])
    tgt = np.concatenate([s, o, self_ids])
    gate = 1.0 / (1.0 + np.exp(-(gfe[idx] + bglab[pr, 0])))
    msg = gate[:, None] * (nf[idx] + blab[pr])
    upd = np.zeros((N, D), np.float32)
    np.add.at(upd, tgt, msg)
    return np.maximum(upd, 0.0)


def _emulate(inputs, plan, in_maps):
    """Numpy emulation of the device program from the STAGED tensors —
    validates host prep + on-device algebra (f32, no bf16 rounding)."""
    def f32(x):
        return np.asarray(x, dtype=np.float32)

    out = np.empty((N, D), np.float32)
    for c in range(NCORES):
        m = {k: f32(v) for k, v in in_maps[c].items()}
        w_k = {kk: m["w"][:, ki * 32 * CW:(ki + 1) * 32 * CW]
               for ki, kk in enumerate((2, 0, 1))}

        def lhsT_to_rows(tile):  # [128, 32*128] -> [128 rows, 4096]
            return tile.reshape(128, 32, 128).transpose(2, 1, 0).reshape(128, D)

        def wmat(kk):  # [128, 32*CW] -> [4096, CW]
            return w_k[kk].reshape(128, 32, CW).transpose(1, 0, 2).reshape(D, CW)

        hp = {}
        for k in (0, 1):
            for pb in range(plan["P"][k]):
                i = pb if k == 0 else plan["P"][0] + pb
                hp[(k, pb)] = lhsT_to_rows(m["prft"][i]) @ wmat(k)
        for b in range(NBLK):
            rows = lhsT_to_rows(m["rft"][b])
            h2 = rows @ wmat(2)
            gf = rows @ (m["wg"].reshape(128, 32, 3).transpose(1, 0, 2)
                         .reshape(D, 3))
            g2 = 1 / (1 + np.exp(-(m["bgb"][:, 0] + gf[:, 2])))
            h2s = g2[:, None] * h2
            pgt = np.zeros((NPRED, 128), np.float32)
            pout = np.zeros((128, CW), np.float32)
            for k in (0, 1):
                sig = 1 / (1 + np.exp(-(m["bgb"] + gf[:, k:k + 1])))
                sl = slice((b * 2 + k) * EPB, (b * 2 + k + 1) * EPB)
                sl128 = slice((b * 2 + k) * 128, (b * 2 + k + 1) * 128)
                prg = m["srct"][:, sl].T @ sig
                pg = prg * m["p1h"][:, b * NPRED:(b + 1) * NPRED]
                pgt += pg.T @ m["tgto"][:, sl128]
                gcol = pg.sum(1)
                sides = [(plan["pb_lo"][(b, k)], m["srcoL"][:, sl128])]
                if plan["pb_hi"][(b, k)] > plan["pb_lo"][(b, k)]:
                    ci = plan["cross_idx"][(b, k)]
                    sides.append((plan["pb_hi"][(b, k)],
                                  m["srcoH"][:, ci * 128:(ci + 1) * 128]))
                for pb, srco in sides:
                    srcg = srco * gcol[:, None]
                    pmt = srcg.T @ m["tgto"][:, sl128]
                    pout += pmt.T @ hp[(k, pb)]
            pg2 = m["p1hs"] * g2[:, None]
            pgt += pg2.T
            pout += pgt.T @ m["blab"]
            out[b * 128:(b + 1) * 128, c * CW:(c + 1) * CW] = np.maximum(
                pout + h2s, 0.0)
    return out


def _run(inputs, trace=False):
    from concourse.bass_utils import run_bass_kernel_spmd

    rels = np.asarray(inputs["rels"])
    plan = _plan_packing(rels)
    key = rels.tobytes()
    if key not in _prog_cache:
        _prog_cache.clear()
        _prog_cache[key] = _build_program(plan)
    nc = _prog_cache[key]
    in_maps = _host_prep(inputs, plan)
    try:
        res = run_bass_kernel_spmd(nc, in_maps, core_ids=list(range(NCORES)),
                                   trace=trace)
    except Exception:
        # transient device errors (e.g. NRT_EXEC_UNIT_UNRECOVERABLE) have
        # been observed to clear on retry
        import time
        time.sleep(5)
        res = run_bass_kernel_spmd(nc, in_maps, core_ids=list(range(NCORES)),
                                   trace=trace)
    out = np.empty((N, D), np.float32)
    for c in range(NCORES):
        out[:, c * CW:(c + 1) * CW] = np.asarray(
            res.results[c]["out"], dtype=np.float32).reshape(N, CW)
    return out, res


def kernel(**inputs):
    if not _rels_are_blocked(inputs["rels"]):
        return _numpy_fallback(inputs)
    out, _ = _run(inputs, trace=False)
    return out


# revision 4
# speedup vs baseline: 1.0374x; 1.0325x over previous
"""GCN message-passing kernel for Trainium2 (8 NeuronCores, SPMD) — v2.

Math (matches the reference):
    gf   = RF @ W_g                          (2048, 3)   gate features
    H_k  = RF @ W_k                          (2048, 4096) per edge type k in {0,1,2}
    gate(e) = sigmoid(gf[src_e, k_e] + b_glab[p_e])
    upd[t]  = sum_{e->t} gate(e) * (H_{k_e}[src_e] + b_lab[p_e])
    out  = relu(upd)

v2 key idea (on top of the v1 block-diagonal gate-matrix formulation):
only ~47% of regions ever appear as a message SOURCE for edge types 0/1
(unique objs/subjs per image ~15 of 32).  H_0/H_1 rows for unused
sources are dead work.  We therefore compute H_0/H_1 only for the
PACKED list of unique sources: the used RF rows are compacted (host
side) into ~8 pseudo-blocks of 128 per edge type instead of 16 full
blocks, cutting the dominant PE matmul stream from 48 to ~32
block-passes (~2/3 of v1's tensor-engine work).  The self-loop type
k=2 still needs every region, and the per-region gate features gf ride
along the k=2 pass as interleaved 3-column matmuls.

Aggregation stays on device: per target block, block-local gate
matrices in PACKED source coordinates are built from one-hot index
matrices (host prepares 0/1 matrices from rels; gates themselves are
computed on Trainium).  A target block's sources span at most 2
pseudo-blocks (sources are packed in region order), so stage3 needs at
most 2 matmuls per edge type.

Sharding: unchanged from v1 — output D dim split 8 ways; each core
computes all 2048 rows x its 512 columns.  No collectives.

The pseudo-block structure depends on `rels`, so the program is
compiled per relation pattern (cached by rels bytes).
"""

import numpy as np
import ml_dtypes

# problem constants (hardcoded per contract)
N_IMG = 64
REG = 32
RPI = 32
NUM_REL = 20
D = 4096
NPRED = 81
N = N_IMG * REG          # 2048
NCORES = 8
CW = D // NCORES         # 512 output cols per core
NBLK = N // 128          # 16 row blocks
IPB = 128 // REG         # 4 images per block
EPB = IPB * NUM_REL      # 80 edges per block per edge type

BF = ml_dtypes.bfloat16
F8 = ml_dtypes.float8_e4m3fn

import os
FP8K0 = os.environ.get("FP8K0", "0") == "1"  # probe: k0 packed passes in fp8
S_R = 32.0     # fp8 scale for RF
S_W = 2048.0   # fp8 scale for W

_prog_cache = {}


def _to_f8(x):
    return np.clip(x, -440.0, 440.0).astype(F8)


def _plan_packing(rels):
    """Host-side packing plan from the relation list.

    Returns a dict with, per edge type k in (0, 1):
      u[k]        sorted unique global source region ids
      P[k]        number of 128-wide pseudo-blocks
      pos[k]      region id -> packed position (array of len N, -1 unused)
    and per (block b, k):
      pb_lo/pb_hi pseudo-blocks containing this block's sources
      cross       whether pb_hi > pb_lo
    plus cross-index assignment for the compact hi-side one-hot tensor.
    """
    rels_r = np.asarray(rels).reshape(N_IMG, RPI, 3)[:, :NUM_REL].reshape(-1, 3)
    s, o = rels_r[:, 1], rels_r[:, 2]
    srcs = {0: o, 1: s}
    plan = {"pb_regs": {}, "P": {}, "pos": {}, "pb_lo": {}, "pb_hi": {},
            "cross_idx": {}, "n_cross": 0, "last_user": {}}
    for k in (0, 1):
        u = np.unique(srcs[k])
        p_dense = -(-len(u) // 128)
        # block-aligned packing: whole target-blocks' source sets per
        # pseudo-block (padded), so no block straddles a pb boundary and
        # stage3 needs exactly one gather matmul per (block, k)
        ub = [np.unique(srcs[k][b * EPB:(b + 1) * EPB]).tolist()
              for b in range(NBLK)]
        pbs, cur = [], []
        for b in range(NBLK):
            if len(cur) + len(ub[b]) <= 128:
                cur.extend(ub[b])
            else:
                pbs.append(cur)
                cur = list(ub[b])
        pbs.append(cur)
        if len(pbs) > p_dense:
            # padding overflowed a pseudo-block: fall back to dense packing
            pbs = [u[i * 128:(i + 1) * 128].tolist() for i in range(p_dense)]
        pos = np.full(N, -1, np.int64)
        for pb, regs in enumerate(pbs):
            pos[np.asarray(regs, np.int64)] = pb * 128 + np.arange(len(regs))
        plan["pb_regs"][k] = pbs
        plan["P"][k] = len(pbs)
        plan["pos"][k] = pos
    n_cross = 0
    for b in range(NBLK):
        for k in (0, 1):
            pk = plan["pos"][k][srcs[k][b * EPB:(b + 1) * EPB]]
            assert np.all(pk >= 0)
            pbs = pk // 128
            lo, hi = int(pbs.min()), int(pbs.max())
            assert hi - lo <= 1, f"block {b} k {k} spans {hi - lo + 1} pbs"
            plan["pb_lo"][(b, k)] = lo
            plan["pb_hi"][(b, k)] = hi
            if hi > lo:
                plan["cross_idx"][(b, k)] = n_cross
                n_cross += 1
            for pb in (lo, hi):
                key = (k, pb)
                plan["last_user"][key] = max(plan["last_user"].get(key, -1), b)
    plan["n_cross"] = max(n_cross, 1)
    return plan





def _build_program(plan):
    import concourse.bass as bass
    import concourse.tile as tile
    from concourse import bacc, mybir

    bf16 = mybir.dt.bfloat16
    f32 = mybir.dt.float32
    AF = mybir.ActivationFunctionType
    ALU = mybir.AluOpType

    P0, P1 = plan["P"][0], plan["P"][1]
    NCROSS = plan["n_cross"]

    nc = bacc.Bacc("TRN2", target_bir_lowering=False, debug=False,
                   num_devices=NCORES)

    f8e4 = mybir.dt.float8e4

    rft = nc.dram_tensor("rft", [NBLK, 128, 32 * 128], bf16, kind="ExternalInput").ap()
    prft = nc.dram_tensor("prft", [P0 + P1, 128, 32 * 128], bf16, kind="ExternalInput").ap()
    if FP8K0:
        prft8 = nc.dram_tensor("prft8", [P0, 128, 32 * 128], f8e4,
                               kind="ExternalInput").ap()
        w08 = nc.dram_tensor("w08", [128, 32 * CW], f8e4,
                             kind="ExternalInput").ap()
    # w layout: k-order (2, 0, 1), i.e. cols [k2 | k0 | k1], chunk-major inside
    w = nc.dram_tensor("w", [128, 3 * 32 * CW], bf16, kind="ExternalInput").ap()
    wg = nc.dram_tensor("wg", [128, 32 * 3], bf16, kind="ExternalInput").ap()
    blab = nc.dram_tensor("blab", [NPRED, CW], bf16, kind="ExternalInput").ap()
    bgb = nc.dram_tensor("bgb", [128, NPRED], bf16, kind="ExternalInput").ap()
    srct = nc.dram_tensor("srct", [128, NBLK * 2 * EPB], bf16, kind="ExternalInput").ap()
    srcoL = nc.dram_tensor("srcoL", [EPB, NBLK * 2 * 128], bf16, kind="ExternalInput").ap()
    srcoH = nc.dram_tensor("srcoH", [EPB, NCROSS * 128], bf16, kind="ExternalInput").ap()
    tgto = nc.dram_tensor("tgto", [EPB, NBLK * 2 * 128], bf16, kind="ExternalInput").ap()
    p1h = nc.dram_tensor("p1h", [EPB, NBLK * NPRED], bf16, kind="ExternalInput").ap()
    p1hs = nc.dram_tensor("p1hs", [128, NPRED], bf16, kind="ExternalInput").ap()
    ident = nc.dram_tensor("ident", [128, 128], bf16, kind="ExternalInput").ap()
    out = nc.dram_tensor("out", [NBLK, 128, CW], bf16, kind="ExternalOutput").ap()

    DEPTH = 6  # k2 blocks run before the first packed pass (absorbs W DMA);
    # only 4 rft tiles are loaded upfront — blocks 4/5 come via prefetch so
    # no startup DMA ever WAR-blocks on pool buffer recycling
    with tile.TileContext(nc) as tc:
        with (
            tc.tile_pool(name="consts", bufs=1) as cpool,
            tc.tile_pool(name="rft", bufs=4) as rpool,
            tc.tile_pool(name="prft", bufs=2) as ppool,
            tc.tile_pool(name="hp", bufs=5) as hppool,
            tc.tile_pool(name="deep", bufs=9) as dpool,
            tc.tile_pool(name="small", bufs=2) as spool,
            tc.tile_pool(name="osb", bufs=2) as opool,
            tc.tile_pool(name="ph", bufs=2, space="PSUM") as php,
            tc.tile_pool(name="pgf", bufs=1, space="PSUM") as pgfp,
            tc.tile_pool(name="prg", bufs=1, space="PSUM") as prgp,
            tc.tile_pool(name="pgt", bufs=1, space="PSUM") as pgtp,
            tc.tile_pool(name="pmt", bufs=2, space="PSUM") as pmtp,
            tc.tile_pool(name="pout", bufs=1, space="PSUM") as poutp,
        ):
            # --- input DMAs, ordered by when the PE needs each tensor ---
            wg_sb = cpool.tile([128, 32 * 3], bf16, tag="wg")
            nc.sync.dma_start(out=wg_sb[:], in_=wg[:])
            WCH = 4 * CW  # w chunk: 4 d-tiles
            w_ch = {k: [cpool.tile([128, WCH], bf16, tag=f"w{k}c{i}",
                                   name=f"w{k}c{i}")
                        for i in range(8)]
                    for k in ((2, 1) if FP8K0 else (2, 0, 1))}

            def _rft_half(b, h):
                t = rpool.tile([128, 16 * 128], bf16, tag=f"rft{h}",
                               name=f"rft{h}_{b}")
                nc.sync.dma_start(out=t[:],
                                  in_=rft[b, :, h * 16 * 128:(h + 1) * 16 * 128])
                return t

            def _load_rft(b):
                rft_tiles[b] = [_rft_half(b, 0), _rft_half(b, 1)]

            def _prft_half(i, h):
                t = ppool.tile([128, 16 * 128], bf16, tag=f"prft{h}",
                               name=f"prft{h}_{i}")
                nc.sync.dma_start(out=t[:],
                                  in_=prft[i, :, h * 16 * 128:(h + 1) * 16 * 128])
                return t

            def _prft8_half(i, h):
                t = ppool.tile([128, 16, 128], f8e4, tag=f"p8_{h}",
                               name=f"p8_{h}_{i}")
                nc.sync.dma_start(out=t[:],
                                  in_=prft8[i, :, h * 16 * 128:(h + 1) * 16 * 128])
                return t

            def _load_prft(k, pb):
                if FP8K0 and k == 0:
                    prft_tiles[(k, pb)] = [_prft8_half(pb, 0), _prft8_half(pb, 1)]
                else:
                    i = pb if k == 0 else P0 + pb
                    prft_tiles[(k, pb)] = [_prft_half(i, 0), _prft_half(i, 1)]

            if FP8K0:
                w08_sb = cpool.tile([128, 32, CW], f8e4, tag="w08",
                                    name="w08")

            def _load_w(k):
                if FP8K0 and k == 0:
                    nc.sync.dma_start(out=w08_sb[:], in_=w08[:])
                    return
                # dram w layout is k-order (2, 0, 1), chunk-major
                base = {2: 0, 0: 1, 1: 2}[k] * 32 * CW
                for i in range(8):
                    nc.sync.dma_start(out=w_ch[k][i][:],
                                      in_=w[:, base + i * WCH:
                                            base + (i + 1) * WCH])

            rft_tiles, prft_tiles = {}, {}
            nc.sync.dma_start(out=w_ch[2][0][:], in_=w[:, 0:WCH])
            _load_rft(0)
            for i in range(1, 8):
                nc.sync.dma_start(out=w_ch[2][i][:],
                                  in_=w[:, i * WCH:(i + 1) * WCH])
            # small consts needed by build(0) early
            blab_sb = cpool.tile([NPRED, CW], bf16, tag="blab")
            nc.sync.dma_start(out=blab_sb[:], in_=blab[:])
            bgb_sb = cpool.tile([128, NPRED], bf16, tag="bgb")
            nc.sync.dma_start(out=bgb_sb[:], in_=bgb[:])
            p1hs_sb = cpool.tile([128, NPRED], bf16, tag="p1hs")
            nc.sync.dma_start(out=p1hs_sb[:], in_=p1hs[:])
            ident_sb = cpool.tile([128, 128], bf16, tag="ident")
            nc.sync.dma_start(out=ident_sb[:], in_=ident[:])
            _load_rft(1)
            srct_sb = cpool.tile([128, NBLK * 2 * EPB], bf16, tag="srct")
            nc.sync.dma_start(out=srct_sb[:], in_=srct[:])
            p1h_sb = cpool.tile([EPB, NBLK * NPRED], bf16, tag="p1h")
            nc.sync.dma_start(out=p1h_sb[:], in_=p1h[:])
            _load_rft(2)
            srcoL_sb = cpool.tile([EPB, NBLK * 2 * 128], bf16, tag="srcoL")
            nc.sync.dma_start(out=srcoL_sb[:], in_=srcoL[:])
            srcoH_sb = cpool.tile([EPB, NCROSS * 128], bf16, tag="srcoH")
            nc.sync.dma_start(out=srcoH_sb[:], in_=srcoH[:])
            tgto_sb = cpool.tile([EPB, NBLK * 2 * 128], bf16, tag="tgto")
            nc.sync.dma_start(out=tgto_sb[:], in_=tgto[:])
            _load_rft(3)
            _load_prft(0, 0)
            _load_w(0)
            # prft(1,0) and W1 are issued at emission time (heavy pass 2),
            # AFTER the rft4/rft5 prefetches — earliest-deadline order in the
            # in-order DMA queues (they are not needed until heavy pass 8)

            h2_sb, hp_sb, gf_tiles, g2_tiles, mtgt = {}, {}, {}, {}, {}

            def rft_lhsT(b, d):
                return rft_tiles[b][d // 16][:, (d % 16) * 128:(d % 16 + 1) * 128]

            def prft_lhsT(k, pb, d):
                return prft_tiles[(k, pb)][d // 16][:, (d % 16) * 128:(d % 16 + 1) * 128]

            from concourse.tile_rust import add_dep_helper

            def k2gf_pass(b):
                """H_2(b) = RF_b @ W_2 with gf(b) interleaved (gf matmuls
                reuse the H matmul's stationary operand via ldweights=False;
                the local ordering chain keeps each gf adjacent to its H
                partner).  The self-loop gate is folded into the PSUM->SBUF
                copy: h2s = diag(g2) @ H_2."""
                ph_t = php.tile([128, CW], f32, tag="ph", name=f"ph{b}_2")
                pgf_t = pgfp.tile([128, 3], f32, tag="pgf", name=f"pgf{b}")
                prev = None
                for d in range(32):
                    lhsT = rft_lhsT(b, d)
                    nc.tensor.matmul(ph_t[:], lhsT,
                                     w_ch[2][d // 4][:, (d % 4) * CW:(d % 4 + 1) * CW],
                                     start=(d == 0), stop=(d == 31))
                    h_inst = nc.main_func.blocks[-1].instructions[-1]
                    assert h_inst.opcode == "Matmult"
                    if prev is not None:
                        add_dep_helper(h_inst, prev, sync=False,
                                       reason="k2-chain")
                    nc.tensor.matmul(pgf_t[:], lhsT,
                                     wg_sb[:, d * 3:(d + 1) * 3],
                                     start=(d == 0), stop=(d == 31))
                    gf_inst = nc.main_func.blocks[-1].instructions[-1]
                    assert gf_inst.opcode == "Matmult"
                    gf_inst.ldweights = False
                    add_dep_helper(gf_inst, h_inst, sync=False,
                                   reason="k2-pair")
                    prev = gf_inst
                gf_sb = dpool.tile([128, 3], f32, tag="gf", name=f"gf{b}")
                nc.vector.tensor_copy(out=gf_sb[:], in_=pgf_t[:])
                gf_tiles[b] = gf_sb
                g2 = dpool.tile([128, 1], f32, tag="g2", name=f"g2_{b}")
                nc.scalar.activation(g2[:], bgb_sb[:, 0:1], AF.Sigmoid,
                                     bias=gf_sb[:, 2:3])
                g2_tiles[b] = g2
                hk = dpool.tile([128, CW], bf16, tag="h2", name=f"h2_{b}")
                nc.vector.tensor_scalar_mul(hk[:], ph_t[:], g2[:])
                h2_sb[b] = hk

            def ppass(k, pb):
                """Packed H_k rows for pseudo-block pb."""
                ph_t = php.tile([128, CW], f32, tag="ph", name=f"php{k}_{pb}")
                if FP8K0 and k == 0:
                    # fp8 DoubleRow: chunk-pair (K=256) per matmul; lhsT free
                    # [2,128] and rhs free [2,512] carry an explicit pair dim
                    for dp in range(16):
                        half = prft_tiles[(k, pb)][dp // 8]
                        j = dp % 8
                        lhsT = half[:, 2 * j:2 * j + 2, :]
                        rhs = w08_sb[:, 2 * dp:2 * dp + 2, :]
                        nc.tensor.matmul(
                            ph_t[:], lhsT, rhs, start=(dp == 0),
                            stop=(dp == 15),
                            perf_mode=mybir.MatmulPerfMode.DoubleRow)
                else:
                    for d in range(32):
                        nc.tensor.matmul(ph_t[:], prft_lhsT(k, pb, d),
                                         w_ch[k][d // 4][:, (d % 4) * CW:(d % 4 + 1) * CW],
                                         start=(d == 0), stop=(d == 31))
                hk = hppool.tile([128, CW], bf16, tag=f"hp{k}",
                                 name=f"hp{k}_{pb}")
                nc.vector.tensor_copy(out=hk[:], in_=ph_t[:])
                hp_sb[(k, pb)] = hk
                del prft_tiles[(k, pb)]

            def build(b):
                """Gates -> packed-coordinate gate matrices (lhsT form,
                [packed_pos_in_pb, target]) and G^T for the b_lab term."""
                gf_sb = gf_tiles[b]
                sig = []
                for k in range(2):
                    sg = spool.tile([128, NPRED], bf16, tag=f"sig{k}",
                                    name=f"sig{b}_{k}")
                    nc.scalar.activation(sg[:], bgb_sb[:], AF.Sigmoid,
                                         bias=gf_sb[:, k:k + 1])
                    sig.append(sg)

                mt_sb = dpool.tile([128, 4 * 128], bf16, tag="mt",
                                   name=f"mt{b}")
                pgt_t = pgtp.tile([NPRED, 128], f32, tag="pgt", name=f"pgt{b}")
                parts = []
                for k in range(2):
                    prg_t = prgp.tile([EPB, NPRED], f32, tag="prg",
                                      name=f"prg{b}_{k}")
                    nc.tensor.matmul(
                        prg_t[:],
                        srct_sb[:, (b * 2 + k) * EPB:(b * 2 + k + 1) * EPB],
                        sig[k][:], start=True, stop=True)
                    pg = spool.tile([EPB, NPRED], bf16, tag="pg",
                                    name=f"pg{b}_{k}")
                    nc.vector.tensor_mul(
                        pg[:], prg_t[:],
                        p1h_sb[:, b * NPRED:(b + 1) * NPRED])
                    nc.tensor.matmul(
                        pgt_t[:], pg[:],
                        tgto_sb[:, (b * 2 + k) * 128:(b * 2 + k + 1) * 128],
                        start=(k == 0), stop=False)
                    gcol = spool.tile([EPB, 1], f32, tag="gcol",
                                      name=f"gcol{b}_{k}")
                    nc.vector.tensor_reduce(gcol[:], pg[:],
                                            axis=mybir.AxisListType.X,
                                            op=ALU.add)
                    sides = [("L", plan["pb_lo"][(b, k)],
                              srcoL_sb[:, (b * 2 + k) * 128:(b * 2 + k + 1) * 128])]
                    if plan["pb_hi"][(b, k)] > plan["pb_lo"][(b, k)]:
                        ci = plan["cross_idx"][(b, k)]
                        sides.append(("H", plan["pb_hi"][(b, k)],
                                      srcoH_sb[:, ci * 128:(ci + 1) * 128]))
                    for si, (nmside, pb, srco_ap) in enumerate(sides):
                        slot = k * 2 + si
                        srcg = spool.tile([EPB, 128], bf16, tag="srcg",
                                          name=f"srcg{b}_{k}{nmside}")
                        nc.vector.tensor_scalar_mul(srcg[:], srco_ap, gcol[:])
                        pmt_t = pmtp.tile([128, 128], f32, tag="pmt",
                                          name=f"pmt{b}_{k}{nmside}")
                        nc.tensor.matmul(
                            pmt_t[:], srcg[:],
                            tgto_sb[:, (b * 2 + k) * 128:(b * 2 + k + 1) * 128],
                            start=True, stop=True)
                        nc.vector.tensor_copy(
                            out=mt_sb[:, slot * 128:(slot + 1) * 128],
                            in_=pmt_t[:])
                        parts.append((k, slot, pb))
                # self-loop: G row 0 += g2
                pg2 = spool.tile([128, NPRED], bf16, tag="pg2", name=f"pg2_{b}")
                nc.vector.tensor_scalar_mul(pg2[:], p1hs_sb[:], g2_tiles[b][:])
                nc.tensor.matmul(pgt_t[:], pg2[:], ident_sb[:],
                                 start=False, stop=True)
                gt_sb = dpool.tile([NPRED, 128], bf16, tag="gt", name=f"gt{b}")
                nc.vector.tensor_copy(out=gt_sb[:], in_=pgt_t[:])
                mtgt[b] = (mt_sb, gt_sb, parts)

            def stage3(b):
                mt_sb, gt_sb, parts = mtgt[b]
                pout_t = poutp.tile([128, CW], f32, tag="pout", name=f"po{b}")
                for i, (k, slot, pb) in enumerate(parts):
                    nc.tensor.matmul(pout_t[:],
                                     mt_sb[:, slot * 128:(slot + 1) * 128],
                                     hp_sb[(k, pb)][:],
                                     start=(i == 0), stop=False)
                nc.tensor.matmul(pout_t[:], gt_sb[:], blab_sb[:],
                                 start=False, stop=True)
                nc.vector.tensor_add(pout_t[:], pout_t[:], h2_sb[b][:])
                out_sb = opool.tile([128, CW], bf16, tag="out", name=f"ob{b}")
                nc.scalar.activation(out_sb[:], pout_t[:], AF.Relu)
                nc.sync.dma_start(out=out[b], in_=out_sb[:])
                del h2_sb[b], gf_tiles[b], g2_tiles[b], mtgt[b]
                del rft_tiles[b]
                for k in (0, 1):
                    for pb in {plan["pb_lo"][(b, k)], plan["pb_hi"][(b, k)]}:
                        if plan["last_user"][(k, pb)] == b:
                            del hp_sb[(k, pb)]

            # --- schedule: "heavy" passes = 16 k2gf + P0+P1 packed, with a
            # DEPTH-block k2 head start to absorb the W0/W1 DMA; packed
            # passes then alternate with the remaining k2 blocks.  builds
            # trail their k2gf block by one heavy pass; stage3(b) is emitted
            # as soon as its packed dependencies have been emitted. ---
            heavy = [("k2", b) for b in range(DEPTH)]
            pq = []
            for i in range(max(P0, P1)):
                if i < P0:
                    pq.append((0, i))
                if i < P1:
                    pq.append((1, i))
            bq = list(range(DEPTH, NBLK))
            while pq or bq:
                if pq:
                    heavy.append(("pp", pq.pop(0)))
                if bq:
                    heavy.append(("k2", bq.pop(0)))

            emitted_pb = {0: 0, 1: 0}
            built = set()
            done_upto = [0]  # next stage3 block

            def try_stage3():
                while (done_upto[0] < NBLK and done_upto[0] in built
                       and all(plan["pb_hi"][(done_upto[0], k)] < emitted_pb[k]
                               for k in (0, 1))):
                    stage3(done_upto[0])
                    done_upto[0] += 1

            pending_build = []
            prefetch_rft = 4     # rft blocks already requested upfront
            prefetch_pp = 2      # packed tiles already requested (1 per k)

            for hi, (kind, arg) in enumerate(heavy):
                if kind == "k2":
                    k2gf_pass(arg)
                    pending_build.append(arg)
                else:
                    k, pb = arg
                    ppass(k, pb)
                    emitted_pb[k] = pb + 1
                # deadline-ordered late loads: rft4/5 recycle buffers of
                # blocks 0/1 (WAR resolves after their k2gf), then W1+prft10
                if hi == 0:
                    _load_rft(4)
                    prefetch_rft = 5
                elif hi == 1:
                    _load_rft(5)
                    prefetch_rft = 6
                elif hi == 2:
                    _load_prft(1, 0)
                    _load_w(1)
                # prefetch DMA for upcoming heavy passes (~3 ahead)
                for j in range(hi + 1, min(hi + 4, len(heavy))):
                    kj, aj = heavy[j]
                    if kj == "k2" and aj >= prefetch_rft:
                        for bb in range(prefetch_rft, aj + 1):
                            _load_rft(bb)
                        prefetch_rft = aj + 1
                    if kj == "pp" and aj not in prft_tiles and aj[1] >= (
                            emitted_pb[aj[0]]):
                        if aj not in prft_tiles:
                            _load_prft(*aj)
                # builds trail by one heavy pass
                while len(pending_build) > 1:
                    bb = pending_build.pop(0)
                    build(bb)
                    built.add(bb)
                try_stage3()
            while pending_build:
                bb = pending_build.pop(0)
                build(bb)
                built.add(bb)
            try_stage3()
            assert done_upto[0] == NBLK, f"only {done_upto[0]} blocks done"

    nc.compile()
    return nc


def _host_prep(inputs, plan):
    rf = np.asarray(inputs["region_feats"], dtype=np.float32)
    W = np.asarray(inputs["W_conv"], dtype=np.float32)
    Wg = np.asarray(inputs["W_g"], dtype=np.float32)
    blab = np.asarray(inputs["b_lab"], dtype=np.float32)
    bglab = np.asarray(inputs["b_glab"], dtype=np.float32)
    rels = np.asarray(inputs["rels"])
    preds = np.asarray(inputs["pred_classes"])

    rels_r = rels.reshape(N_IMG, RPI, 3)[:, :NUM_REL].reshape(-1, 3)
    preds_r = preds.reshape(N_IMG, RPI)[:, :NUM_REL].reshape(-1)

    # RF^T tiles: rft_h[b, p, d*128+j] = RF[b*128+j, d*128+p]
    rft_h = np.ascontiguousarray(
        rf.T.reshape(32, 128, NBLK, 128).transpose(2, 1, 0, 3), dtype=BF
    ).reshape(NBLK, 128, 32 * 128)

    # packed RF^T tiles per (k, pb)
    P0, P1 = plan["P"][0], plan["P"][1]
    prft_h = np.zeros((P0 + P1, 128, 32 * 128), BF)
    prft8_h = np.zeros((P0, 128, 32 * 128), F8) if FP8K0 else None
    for k in (0, 1):
        for pb in range(plan["P"][k]):
            regs = np.asarray(plan["pb_regs"][k][pb], np.int64)
            m = len(regs)
            # [m, D] -> [D, m] -> [32, 128, m] (d, p, j) -> (p, d, j)
            t = rf[regs].T.reshape(32, 128, m).transpose(1, 0, 2)
            i = pb if k == 0 else P0 + pb
            prft_h[i].reshape(128, 32, 128)[:, :, :m] = t.astype(BF)
            if FP8K0 and k == 0:
                prft8_h[pb].reshape(128, 32, 128)[:, :, :m] = _to_f8(t * S_R)

    # W slices per core, k-order (2, 0, 1):
    # w_h[p, ((ki*32+d)*CW)+j] = W[d*128+p, korder[ki]*D + c*CW + j]
    Wr = W.reshape(32, 128, 3, NCORES, CW)[:, :, (2, 0, 1)]
    w_cores = [
        np.ascontiguousarray(Wr[:, :, :, c, :].transpose(1, 2, 0, 3),
                             dtype=BF).reshape(128, 3 * 32 * CW)
        for c in range(NCORES)
    ]
    if FP8K0:
        # k=0 W slice (index 1 in (2,0,1) order), chunk-major, fp8-scaled
        w08_cores = [
            _to_f8(Wr[:, :, 1, c, :].transpose(1, 0, 2).reshape(128, 32 * CW)
                   * S_W)
            for c in range(NCORES)
        ]
    wg_h = np.ascontiguousarray(
        Wg.reshape(32, 128, 3).transpose(1, 0, 2), dtype=BF
    ).reshape(128, 32 * 3)
    blab_cores = [
        np.ascontiguousarray(blab[:, c * CW:(c + 1) * CW], dtype=BF)
        for c in range(NCORES)
    ]
    bgb_h = np.ascontiguousarray(
        np.repeat(bglab.reshape(1, NPRED), 128, axis=0), dtype=BF)

    srct_h = np.zeros((128, NBLK * 2 * EPB), np.float32)
    srcoL_h = np.zeros((EPB, NBLK * 2 * 128), np.float32)
    srcoH_h = np.zeros((EPB, plan["n_cross"] * 128), np.float32)
    tgto_h = np.zeros((EPB, NBLK * 2 * 128), np.float32)
    p1h_h = np.zeros((EPB, NBLK * NPRED), np.float32)
    e = np.arange(EPB)
    for b in range(NBLK):
        eb = rels_r[b * EPB:(b + 1) * EPB]
        pb_ = preds_r[b * EPB:(b + 1) * EPB]
        s = eb[:, 1] - b * 128
        o = eb[:, 2] - b * 128
        # k=0: obj -> subj (src=o, tgt=s); k=1: subj -> obj (src=s, tgt=o)
        for k, (src_loc, tgt_loc) in enumerate(((o, s), (s, o))):
            # fp8 path: descale factor folded into the packed-src one-hots
            oneval = 1.0 / (S_R * S_W) if (FP8K0 and k == 0) else 1.0
            srct_h[src_loc, (b * 2 + k) * EPB + e] = 1.0
            tgto_h[e, (b * 2 + k) * 128 + tgt_loc] = 1.0
            pk = plan["pos"][k][src_loc + b * 128]
            lo = plan["pb_lo"][(b, k)]
            hi = plan["pb_hi"][(b, k)]
            mlo = (pk // 128) == lo
            srcoL_h[e[mlo], (b * 2 + k) * 128 + (pk[mlo] - lo * 128)] = oneval
            if hi > lo:
                ci = plan["cross_idx"][(b, k)]
                mhi = ~mlo
                srcoH_h[e[mhi], ci * 128 + (pk[mhi] - hi * 128)] = oneval
        p1h_h[e, b * NPRED + pb_] = 1.0
    p1hs_h = np.zeros((128, NPRED), np.float32)
    p1hs_h[:, 0] = 1.0

    shared = {
        "rft": rft_h,
        "prft": prft_h,
        "wg": wg_h,
        "bgb": bgb_h,
        "srct": srct_h.astype(BF),
        "srcoL": srcoL_h.astype(BF),
        "srcoH": srcoH_h.astype(BF),
        "tgto": tgto_h.astype(BF),
        "p1h": p1h_h.astype(BF),
        "p1hs": p1hs_h.astype(BF),
        "ident": np.eye(128, dtype=np.float32).astype(BF),
    }
    if FP8K0:
        shared["prft8"] = prft8_h
    in_maps = []
    for c in range(NCORES):
        m = dict(shared)
        m["w"] = w_cores[c]
        m["blab"] = blab_cores[c]
        if FP8K0:
            m["w08"] = w08_cores[c]
        in_maps.append(m)
    return in_maps


def _rels_are_blocked(rels):
    """Check each image's relations reference only that image's regions."""
    rels = np.asarray(rels)
    if rels.shape != (N_IMG * RPI, 3):
        return False
    rels_r = rels.reshape(N_IMG, RPI, 3)[:, :NUM_REL]
    img = np.arange(N_IMG)[:, None]
    lo, hi = img * REG, (img + 1) * REG
    so = rels_r[:, :, 1:3]
    return bool(np.all((so >= lo[:, :, None]) & (so < hi[:, :, None])))


def _numpy_fallback(inputs):
    """Reference-equivalent host computation (only used if the per-image
    relation structure assumption is violated)."""
    rf = np.asarray(inputs["region_feats"], dtype=np.float32)
    W = np.asarray(inputs["W_conv"], dtype=np.float32)
    Wg = np.asarray(inputs["W_g"], dtype=np.float32)
    blab = np.asarray(inputs["b_lab"], dtype=np.float32)
    bglab = np.asarray(inputs["b_glab"], dtype=np.float32)
    rels = np.asarray(inputs["rels"])
    preds = np.asarray(inputs["pred_classes"])
    rels_r = rels.reshape(N_IMG, RPI, 3)[:, :NUM_REL].reshape(-1, 3)
    preds_r = preds.reshape(N_IMG, RPI)[:, :NUM_REL].reshape(-1)
    nf = (rf @ W).reshape(-1, D)
    gfe = (rf @ Wg).reshape(-1)
    s, o = rels_r[:, 1], rels_r[:, 2]
    self_ids = np.arange(N)
    idx = np.concatenate([o * 3 + 0, s * 3 + 1, self_ids * 3 + 2])
    pr = np.concatenate([preds_r, preds_r, np.zeros(N, preds_r.dtype)])
    tgt = np.concatenate([s, o, self_ids])
    gate = 1.0 / (1.0 + np.exp(-(gfe[idx] + bglab[pr, 0])))
    msg = gate[:, None] * (nf[idx] + blab[pr])
    upd = np.zeros((N, D), np.float32)
    np.add.at(upd, tgt, msg)
    return np.maximum(upd, 0.0)


def _emulate(inputs, plan, in_maps):
    """Numpy emulation of the device program from the STAGED tensors —
    validates host prep + on-device algebra (f32, no bf16 rounding)."""
    def f32(x):
        return np.asarray(x, dtype=np.float32)

    out = np.empty((N, D), np.float32)
    for c in range(NCORES):
        m = {k: f32(v) for k, v in in_maps[c].items()}
        w_k = {kk: m["w"][:, ki * 32 * CW:(ki + 1) * 32 * CW]
               for ki, kk in enumerate((2, 0, 1))}

        def lhsT_to_rows(tile):  # [128, 32*128] -> [128 rows, 4096]
            return tile.reshape(128, 32, 128).transpose(2, 1, 0).reshape(128, D)

        def wmat(kk):  # [128, 32*CW] -> [4096, CW]
            return w_k[kk].reshape(128, 32, CW).transpose(1, 0, 2).reshape(D, CW)

        hp = {}
        for k in (0, 1):
            for pb in range(plan["P"][k]):
                i = pb if k == 0 else plan["P"][0] + pb
                hp[(k, pb)] = lhsT_to_rows(m["prft"][i]) @ wmat(k)
        for b in range(NBLK):
            rows = lhsT_to_rows(m["rft"][b])
            h2 = rows @ wmat(2)
            gf = rows @ (m["wg"].reshape(128, 32, 3).transpose(1, 0, 2)
                         .reshape(D, 3))
            g2 = 1 / (1 + np.exp(-(m["bgb"][:, 0] + gf[:, 2])))
            h2s = g2[:, None] * h2
            pgt = np.zeros((NPRED, 128), np.float32)
            pout = np.zeros((128, CW), np.float32)
            for k in (0, 1):
                sig = 1 / (1 + np.exp(-(m["bgb"] + gf[:, k:k + 1])))
                sl = slice((b * 2 + k) * EPB, (b * 2 + k + 1) * EPB)
                sl128 = slice((b * 2 + k) * 128, (b * 2 + k + 1) * 128)
                prg = m["srct"][:, sl].T @ sig
                pg = prg * m["p1h"][:, b * NPRED:(b + 1) * NPRED]
                pgt += pg.T @ m["tgto"][:, sl128]
                gcol = pg.sum(1)
                sides = [(plan["pb_lo"][(b, k)], m["srcoL"][:, sl128])]
                if plan["pb_hi"][(b, k)] > plan["pb_lo"][(b, k)]:
                    ci = plan["cross_idx"][(b, k)]
                    sides.append((plan["pb_hi"][(b, k)],
                                  m["srcoH"][:, ci * 128:(ci + 1) * 128]))
                for pb, srco in sides:
                    srcg = srco * gcol[:, None]
                    pmt = srcg.T @ m["tgto"][:, sl128]
                    pout += pmt.T @ hp[(k, pb)]
            pg2 = m["p1hs"] * g2[:, None]
            pgt += pg2.T
            pout += pgt.T @ m["blab"]
            out[b * 128:(b + 1) * 128, c * CW:(c + 1) * CW] = np.maximum(
                pout + h2s, 0.0)
    return out


def _run(inputs, trace=False):
    from concourse.bass_utils import run_bass_kernel_spmd

    rels = np.asarray(inputs["rels"])
    plan = _plan_packing(rels)
    key = rels.tobytes()
    if key not in _prog_cache:
        _prog_cache.clear()
        _prog_cache[key] = _build_program(plan)
    nc = _prog_cache[key]
    in_maps = _host_prep(inputs, plan)
    try:
        res = run_bass_kernel_spmd(nc, in_maps, core_ids=list(range(NCORES)),
                                   trace=trace)
    except Exception:
        # transient device errors (e.g. NRT_EXEC_UNIT_UNRECOVERABLE) have
        # been observed to clear on retry
        import time
        time.sleep(5)
        res = run_bass_kernel_spmd(nc, in_maps, core_ids=list(range(NCORES)),
                                   trace=trace)
    out = np.empty((N, D), np.float32)
    for c in range(NCORES):
        out[:, c * CW:(c + 1) * CW] = np.asarray(
            res.results[c]["out"], dtype=np.float32).reshape(N, CW)
    return out, res


def kernel(**inputs):
    if not _rels_are_blocked(inputs["rels"]):
        return _numpy_fallback(inputs)
    out, _ = _run(inputs, trace=False)
    return out


# revision 5
# speedup vs baseline: 1.0554x; 1.0173x over previous
"""GCN message-passing kernel for Trainium2 (8 NeuronCores, SPMD) — v2.

Math (matches the reference):
    gf   = RF @ W_g                          (2048, 3)   gate features
    H_k  = RF @ W_k                          (2048, 4096) per edge type k in {0,1,2}
    gate(e) = sigmoid(gf[src_e, k_e] + b_glab[p_e])
    upd[t]  = sum_{e->t} gate(e) * (H_{k_e}[src_e] + b_lab[p_e])
    out  = relu(upd)

v2 key idea (on top of the v1 block-diagonal gate-matrix formulation):
only ~47% of regions ever appear as a message SOURCE for edge types 0/1
(unique objs/subjs per image ~15 of 32).  H_0/H_1 rows for unused
sources are dead work.  We therefore compute H_0/H_1 only for the
PACKED list of unique sources: the used RF rows are compacted (host
side) into ~8 pseudo-blocks of 128 per edge type instead of 16 full
blocks, cutting the dominant PE matmul stream from 48 to ~32
block-passes (~2/3 of v1's tensor-engine work).  The self-loop type
k=2 still needs every region, and the per-region gate features gf ride
along the k=2 pass as interleaved 3-column matmuls.

Aggregation stays on device: per target block, block-local gate
matrices in PACKED source coordinates are built from one-hot index
matrices (host prepares 0/1 matrices from rels; gates themselves are
computed on Trainium).  A target block's sources span at most 2
pseudo-blocks (sources are packed in region order), so stage3 needs at
most 2 matmuls per edge type.

Sharding: unchanged from v1 — output D dim split 8 ways; each core
computes all 2048 rows x its 512 columns.  No collectives.

The pseudo-block structure depends on `rels`, so the program is
compiled per relation pattern (cached by rels bytes).
"""

import numpy as np
import ml_dtypes

# problem constants (hardcoded per contract)
N_IMG = 64
REG = 32
RPI = 32
NUM_REL = 20
D = 4096
NPRED = 81
N = N_IMG * REG          # 2048
NCORES = 8
CW = D // NCORES         # 512 output cols per core
NBLK = N // 128          # 16 row blocks
IPB = 128 // REG         # 4 images per block
EPB = IPB * NUM_REL      # 80 edges per block per edge type

BF = ml_dtypes.bfloat16
F8 = ml_dtypes.float8_e4m3fn

import os
FP8K0 = os.environ.get("FP8K0", "0") == "1"  # probe: k0 packed passes in fp8
S_R = 32.0     # fp8 scale for RF
S_W = 2048.0   # fp8 scale for W

_prog_cache = {}


def _to_f8(x):
    return np.clip(x, -440.0, 440.0).astype(F8)


def _plan_packing(rels):
    """Host-side packing plan from the relation list.

    Returns a dict with, per edge type k in (0, 1):
      u[k]        sorted unique global source region ids
      P[k]        number of 128-wide pseudo-blocks
      pos[k]      region id -> packed position (array of len N, -1 unused)
    and per (block b, k):
      pb_lo/pb_hi pseudo-blocks containing this block's sources
      cross       whether pb_hi > pb_lo
    plus cross-index assignment for the compact hi-side one-hot tensor.
    """
    rels_r = np.asarray(rels).reshape(N_IMG, RPI, 3)[:, :NUM_REL].reshape(-1, 3)
    s, o = rels_r[:, 1], rels_r[:, 2]
    srcs = {0: o, 1: s}
    plan = {"pb_regs": {}, "P": {}, "pos": {}, "pb_lo": {}, "pb_hi": {},
            "cross_idx": {}, "n_cross": 0, "last_user": {}}
    for k in (0, 1):
        u = np.unique(srcs[k])
        p_dense = -(-len(u) // 128)
        # block-aligned packing: whole target-blocks' source sets per
        # pseudo-block (padded), so no block straddles a pb boundary and
        # stage3 needs exactly one gather matmul per (block, k)
        ub = [np.unique(srcs[k][b * EPB:(b + 1) * EPB]).tolist()
              for b in range(NBLK)]
        pbs, cur = [], []
        for b in range(NBLK):
            if len(cur) + len(ub[b]) <= 128:
                cur.extend(ub[b])
            else:
                pbs.append(cur)
                cur = list(ub[b])
        pbs.append(cur)
        if len(pbs) > p_dense:
            # padding overflowed a pseudo-block: fall back to dense packing
            pbs = [u[i * 128:(i + 1) * 128].tolist() for i in range(p_dense)]
        pos = np.full(N, -1, np.int64)
        for pb, regs in enumerate(pbs):
            pos[np.asarray(regs, np.int64)] = pb * 128 + np.arange(len(regs))
        plan["pb_regs"][k] = pbs
        plan["P"][k] = len(pbs)
        plan["pos"][k] = pos
    n_cross = 0
    for b in range(NBLK):
        for k in (0, 1):
            pk = plan["pos"][k][srcs[k][b * EPB:(b + 1) * EPB]]
            assert np.all(pk >= 0)
            pbs = pk // 128
            lo, hi = int(pbs.min()), int(pbs.max())
            assert hi - lo <= 1, f"block {b} k {k} spans {hi - lo + 1} pbs"
            plan["pb_lo"][(b, k)] = lo
            plan["pb_hi"][(b, k)] = hi
            if hi > lo:
                plan["cross_idx"][(b, k)] = n_cross
                n_cross += 1
            for pb in (lo, hi):
                key = (k, pb)
                plan["last_user"][key] = max(plan["last_user"].get(key, -1), b)
    plan["n_cross"] = max(n_cross, 1)
    return plan





def _build_program(plan):
    import concourse.bass as bass
    import concourse.tile as tile
    from concourse import bacc, mybir

    bf16 = mybir.dt.bfloat16
    f32 = mybir.dt.float32
    AF = mybir.ActivationFunctionType
    ALU = mybir.AluOpType

    P0, P1 = plan["P"][0], plan["P"][1]
    NCROSS = plan["n_cross"]

    nc = bacc.Bacc("TRN2", target_bir_lowering=False, debug=False,
                   num_devices=NCORES)

    f8e4 = mybir.dt.float8e4

    rft = nc.dram_tensor("rft", [NBLK, 128, 32 * 128], bf16, kind="ExternalInput").ap()
    prft = nc.dram_tensor("prft", [P0 + P1, 128, 32 * 128], bf16, kind="ExternalInput").ap()
    if FP8K0:
        prft8 = nc.dram_tensor("prft8", [P0, 128, 32 * 128], f8e4,
                               kind="ExternalInput").ap()
        w08 = nc.dram_tensor("w08", [128, 32 * CW], f8e4,
                             kind="ExternalInput").ap()
    # w layout: k-order (2, 0, 1), i.e. cols [k2 | k0 | k1], chunk-major inside
    w = nc.dram_tensor("w", [128, 3 * 32 * CW], bf16, kind="ExternalInput").ap()
    wg = nc.dram_tensor("wg", [128, 32 * 3], bf16, kind="ExternalInput").ap()
    blab = nc.dram_tensor("blab", [NPRED, CW], bf16, kind="ExternalInput").ap()
    bgb = nc.dram_tensor("bgb", [128, NPRED], bf16, kind="ExternalInput").ap()
    srct = nc.dram_tensor("srct", [128, NBLK * 2 * EPB], bf16, kind="ExternalInput").ap()
    srcoL = nc.dram_tensor("srcoL", [EPB, NBLK * 2 * 128], bf16, kind="ExternalInput").ap()
    srcoH = nc.dram_tensor("srcoH", [EPB, NCROSS * 128], bf16, kind="ExternalInput").ap()
    tgto = nc.dram_tensor("tgto", [EPB, NBLK * 2 * 128], bf16, kind="ExternalInput").ap()
    p1h = nc.dram_tensor("p1h", [EPB, NBLK * NPRED], bf16, kind="ExternalInput").ap()
    p1hs = nc.dram_tensor("p1hs", [128, NPRED], bf16, kind="ExternalInput").ap()
    ident = nc.dram_tensor("ident", [128, 128], bf16, kind="ExternalInput").ap()
    out = nc.dram_tensor("out", [NBLK, 128, CW], bf16, kind="ExternalOutput").ap()

    DEPTH = 7  # k2 blocks run before the first packed pass (absorbs W DMA);
    # only 4 rft tiles are loaded upfront — blocks 4/5 come via prefetch so
    # no startup DMA ever WAR-blocks on pool buffer recycling
    with tile.TileContext(nc) as tc:
        with (
            tc.tile_pool(name="consts", bufs=1) as cpool,
            tc.tile_pool(name="rft", bufs=4) as rpool,
            tc.tile_pool(name="prft", bufs=2) as ppool,
            tc.tile_pool(name="hp", bufs=5) as hppool,
            tc.tile_pool(name="deep", bufs=9) as dpool,
            tc.tile_pool(name="small", bufs=2) as spool,
            tc.tile_pool(name="osb", bufs=2) as opool,
            tc.tile_pool(name="ph", bufs=2, space="PSUM") as php,
            tc.tile_pool(name="pgf", bufs=1, space="PSUM") as pgfp,
            tc.tile_pool(name="prg", bufs=1, space="PSUM") as prgp,
            tc.tile_pool(name="pgt", bufs=1, space="PSUM") as pgtp,
            tc.tile_pool(name="pmt", bufs=2, space="PSUM") as pmtp,
            tc.tile_pool(name="pout", bufs=1, space="PSUM") as poutp,
        ):
            # --- input DMAs, ordered by when the PE needs each tensor ---
            wg_sb = cpool.tile([128, 32 * 3], bf16, tag="wg")
            nc.sync.dma_start(out=wg_sb[:], in_=wg[:])
            WCH = 4 * CW  # w chunk: 4 d-tiles
            w_ch = {k: [cpool.tile([128, WCH], bf16, tag=f"w{k}c{i}",
                                   name=f"w{k}c{i}")
                        for i in range(8)]
                    for k in ((2, 1) if FP8K0 else (2, 0, 1))}

            def _rft_half(b, h):
                t = rpool.tile([128, 16 * 128], bf16, tag=f"rft{h}",
                               name=f"rft{h}_{b}")
                nc.sync.dma_start(out=t[:],
                                  in_=rft[b, :, h * 16 * 128:(h + 1) * 16 * 128])
                return t

            def _load_rft(b):
                rft_tiles[b] = [_rft_half(b, 0), _rft_half(b, 1)]

            def _prft_half(i, h):
                t = ppool.tile([128, 16 * 128], bf16, tag=f"prft{h}",
                               name=f"prft{h}_{i}")
                nc.sync.dma_start(out=t[:],
                                  in_=prft[i, :, h * 16 * 128:(h + 1) * 16 * 128])
                return t

            def _prft8_half(i, h):
                t = ppool.tile([128, 16, 128], f8e4, tag=f"p8_{h}",
                               name=f"p8_{h}_{i}")
                nc.sync.dma_start(out=t[:],
                                  in_=prft8[i, :, h * 16 * 128:(h + 1) * 16 * 128])
                return t

            def _load_prft(k, pb):
                if FP8K0 and k == 0:
                    prft_tiles[(k, pb)] = [_prft8_half(pb, 0), _prft8_half(pb, 1)]
                else:
                    i = pb if k == 0 else P0 + pb
                    prft_tiles[(k, pb)] = [_prft_half(i, 0), _prft_half(i, 1)]

            if FP8K0:
                w08_sb = cpool.tile([128, 32, CW], f8e4, tag="w08",
                                    name="w08")

            def _load_w(k):
                if FP8K0 and k == 0:
                    nc.sync.dma_start(out=w08_sb[:], in_=w08[:])
                    return
                # dram w layout is k-order (2, 0, 1), chunk-major
                base = {2: 0, 0: 1, 1: 2}[k] * 32 * CW
                for i in range(8):
                    nc.sync.dma_start(out=w_ch[k][i][:],
                                      in_=w[:, base + i * WCH:
                                            base + (i + 1) * WCH])

            rft_tiles, prft_tiles = {}, {}
            nc.sync.dma_start(out=w_ch[2][0][:], in_=w[:, 0:WCH])
            _load_rft(0)
            for i in range(1, 8):
                nc.sync.dma_start(out=w_ch[2][i][:],
                                  in_=w[:, i * WCH:(i + 1) * WCH])
            # small consts needed by build(0) early
            blab_sb = cpool.tile([NPRED, CW], bf16, tag="blab")
            nc.sync.dma_start(out=blab_sb[:], in_=blab[:])
            bgb_sb = cpool.tile([128, NPRED], bf16, tag="bgb")
            nc.sync.dma_start(out=bgb_sb[:], in_=bgb[:])
            p1hs_sb = cpool.tile([128, NPRED], bf16, tag="p1hs")
            nc.sync.dma_start(out=p1hs_sb[:], in_=p1hs[:])
            ident_sb = cpool.tile([128, 128], bf16, tag="ident")
            nc.sync.dma_start(out=ident_sb[:], in_=ident[:])
            _load_rft(1)
            srct_sb = cpool.tile([128, NBLK * 2 * EPB], bf16, tag="srct")
            nc.sync.dma_start(out=srct_sb[:], in_=srct[:])
            p1h_sb = cpool.tile([EPB, NBLK * NPRED], bf16, tag="p1h")
            nc.sync.dma_start(out=p1h_sb[:], in_=p1h[:])
            _load_rft(2)
            srcoL_sb = cpool.tile([EPB, NBLK * 2 * 128], bf16, tag="srcoL")
            nc.sync.dma_start(out=srcoL_sb[:], in_=srcoL[:])
            srcoH_sb = cpool.tile([EPB, NCROSS * 128], bf16, tag="srcoH")
            nc.sync.dma_start(out=srcoH_sb[:], in_=srcoH[:])
            tgto_sb = cpool.tile([EPB, NBLK * 2 * 128], bf16, tag="tgto")
            nc.sync.dma_start(out=tgto_sb[:], in_=tgto[:])
            _load_rft(3)
            _load_prft(0, 0)
            # W0, W1 and prft(1,0) are issued at emission time, AFTER the
            # rft4/5/6 prefetches — earliest-deadline order in the in-order
            # DMA queues (packed passes start only at heavy pass 7/9)

            h2_sb, hp_sb, gf_tiles, g2_tiles, mtgt = {}, {}, {}, {}, {}

            def rft_lhsT(b, d):
                return rft_tiles[b][d // 16][:, (d % 16) * 128:(d % 16 + 1) * 128]

            def prft_lhsT(k, pb, d):
                return prft_tiles[(k, pb)][d // 16][:, (d % 16) * 128:(d % 16 + 1) * 128]

            from concourse.tile_rust import add_dep_helper

            def k2gf_pass(b):
                """H_2(b) = RF_b @ W_2 with gf(b) interleaved (gf matmuls
                reuse the H matmul's stationary operand via ldweights=False;
                the local ordering chain keeps each gf adjacent to its H
                partner).  The self-loop gate is folded into the PSUM->SBUF
                copy: h2s = diag(g2) @ H_2."""
                ph_t = php.tile([128, CW], f32, tag="ph", name=f"ph{b}_2")
                pgf_t = pgfp.tile([128, 3], f32, tag="pgf", name=f"pgf{b}")
                prev = None
                for d in range(32):
                    lhsT = rft_lhsT(b, d)
                    nc.tensor.matmul(ph_t[:], lhsT,
                                     w_ch[2][d // 4][:, (d % 4) * CW:(d % 4 + 1) * CW],
                                     start=(d == 0), stop=(d == 31))
                    h_inst = nc.main_func.blocks[-1].instructions[-1]
                    assert h_inst.opcode == "Matmult"
                    if prev is not None:
                        add_dep_helper(h_inst, prev, sync=False,
                                       reason="k2-chain")
                    nc.tensor.matmul(pgf_t[:], lhsT,
                                     wg_sb[:, d * 3:(d + 1) * 3],
                                     start=(d == 0), stop=(d == 31))
                    gf_inst = nc.main_func.blocks[-1].instructions[-1]
                    assert gf_inst.opcode == "Matmult"
                    gf_inst.ldweights = False
                    add_dep_helper(gf_inst, h_inst, sync=False,
                                   reason="k2-pair")
                    prev = gf_inst
                gf_sb = dpool.tile([128, 3], f32, tag="gf", name=f"gf{b}")
                nc.vector.tensor_copy(out=gf_sb[:], in_=pgf_t[:])
                gf_tiles[b] = gf_sb
                g2 = dpool.tile([128, 1], f32, tag="g2", name=f"g2_{b}")
                nc.scalar.activation(g2[:], bgb_sb[:, 0:1], AF.Sigmoid,
                                     bias=gf_sb[:, 2:3])
                g2_tiles[b] = g2
                hk = dpool.tile([128, CW], bf16, tag="h2", name=f"h2_{b}")
                nc.vector.tensor_scalar_mul(hk[:], ph_t[:], g2[:])
                h2_sb[b] = hk

            def ppass(k, pb):
                """Packed H_k rows for pseudo-block pb."""
                ph_t = php.tile([128, CW], f32, tag="ph", name=f"php{k}_{pb}")
                if FP8K0 and k == 0:
                    # fp8 DoubleRow: chunk-pair (K=256) per matmul; lhsT free
                    # [2,128] and rhs free [2,512] carry an explicit pair dim
                    for dp in range(16):
                        half = prft_tiles[(k, pb)][dp // 8]
                        j = dp % 8
                        lhsT = half[:, 2 * j:2 * j + 2, :]
                        rhs = w08_sb[:, 2 * dp:2 * dp + 2, :]
                        nc.tensor.matmul(
                            ph_t[:], lhsT, rhs, start=(dp == 0),
                            stop=(dp == 15),
                            perf_mode=mybir.MatmulPerfMode.DoubleRow)
                else:
                    for d in range(32):
                        nc.tensor.matmul(ph_t[:], prft_lhsT(k, pb, d),
                                         w_ch[k][d // 4][:, (d % 4) * CW:(d % 4 + 1) * CW],
                                         start=(d == 0), stop=(d == 31))
                hk = hppool.tile([128, CW], bf16, tag=f"hp{k}",
                                 name=f"hp{k}_{pb}")
                nc.vector.tensor_copy(out=hk[:], in_=ph_t[:])
                hp_sb[(k, pb)] = hk
                del prft_tiles[(k, pb)]

            def build(b):
                """Gates -> packed-coordinate gate matrices (lhsT form,
                [packed_pos_in_pb, target]) and G^T for the b_lab term."""
                gf_sb = gf_tiles[b]
                sig = []
                for k in range(2):
                    sg = spool.tile([128, NPRED], bf16, tag=f"sig{k}",
                                    name=f"sig{b}_{k}")
                    nc.scalar.activation(sg[:], bgb_sb[:], AF.Sigmoid,
                                         bias=gf_sb[:, k:k + 1])
                    sig.append(sg)

                mt_sb = dpool.tile([128, 4 * 128], bf16, tag="mt",
                                   name=f"mt{b}")
                pgt_t = pgtp.tile([NPRED, 128], f32, tag="pgt", name=f"pgt{b}")
                parts = []
                for k in range(2):
                    prg_t = prgp.tile([EPB, NPRED], f32, tag="prg",
                                      name=f"prg{b}_{k}")
                    nc.tensor.matmul(
                        prg_t[:],
                        srct_sb[:, (b * 2 + k) * EPB:(b * 2 + k + 1) * EPB],
                        sig[k][:], start=True, stop=True)
                    pg = spool.tile([EPB, NPRED], bf16, tag="pg",
                                    name=f"pg{b}_{k}")
                    nc.vector.tensor_mul(
                        pg[:], prg_t[:],
                        p1h_sb[:, b * NPRED:(b + 1) * NPRED])
                    nc.tensor.matmul(
                        pgt_t[:], pg[:],
                        tgto_sb[:, (b * 2 + k) * 128:(b * 2 + k + 1) * 128],
                        start=(k == 0), stop=False)
                    gcol = spool.tile([EPB, 1], f32, tag="gcol",
                                      name=f"gcol{b}_{k}")
                    nc.vector.tensor_reduce(gcol[:], pg[:],
                                            axis=mybir.AxisListType.X,
                                            op=ALU.add)
                    sides = [("L", plan["pb_lo"][(b, k)],
                              srcoL_sb[:, (b * 2 + k) * 128:(b * 2 + k + 1) * 128])]
                    if plan["pb_hi"][(b, k)] > plan["pb_lo"][(b, k)]:
                        ci = plan["cross_idx"][(b, k)]
                        sides.append(("H", plan["pb_hi"][(b, k)],
                                      srcoH_sb[:, ci * 128:(ci + 1) * 128]))
                    for si, (nmside, pb, srco_ap) in enumerate(sides):
                        slot = k * 2 + si
                        srcg = spool.tile([EPB, 128], bf16, tag="srcg",
                                          name=f"srcg{b}_{k}{nmside}")
                        nc.vector.tensor_scalar_mul(srcg[:], srco_ap, gcol[:])
                        pmt_t = pmtp.tile([128, 128], f32, tag="pmt",
                                          name=f"pmt{b}_{k}{nmside}")
                        nc.tensor.matmul(
                            pmt_t[:], srcg[:],
                            tgto_sb[:, (b * 2 + k) * 128:(b * 2 + k + 1) * 128],
                            start=True, stop=True)
                        nc.vector.tensor_copy(
                            out=mt_sb[:, slot * 128:(slot + 1) * 128],
                            in_=pmt_t[:])
                        parts.append((k, slot, pb))
                # self-loop: G row 0 += g2
                pg2 = spool.tile([128, NPRED], bf16, tag="pg2", name=f"pg2_{b}")
                nc.vector.tensor_scalar_mul(pg2[:], p1hs_sb[:], g2_tiles[b][:])
                nc.tensor.matmul(pgt_t[:], pg2[:], ident_sb[:],
                                 start=False, stop=True)
                gt_sb = dpool.tile([NPRED, 128], bf16, tag="gt", name=f"gt{b}")
                nc.vector.tensor_copy(out=gt_sb[:], in_=pgt_t[:])
                mtgt[b] = (mt_sb, gt_sb, parts)

            def stage3(b):
                mt_sb, gt_sb, parts = mtgt[b]
                pout_t = poutp.tile([128, CW], f32, tag="pout", name=f"po{b}")
                for i, (k, slot, pb) in enumerate(parts):
                    nc.tensor.matmul(pout_t[:],
                                     mt_sb[:, slot * 128:(slot + 1) * 128],
                                     hp_sb[(k, pb)][:],
                                     start=(i == 0), stop=False)
                nc.tensor.matmul(pout_t[:], gt_sb[:], blab_sb[:],
                                 start=False, stop=True)
                nc.vector.tensor_add(pout_t[:], pout_t[:], h2_sb[b][:])
                out_sb = opool.tile([128, CW], bf16, tag="out", name=f"ob{b}")
                nc.scalar.activation(out_sb[:], pout_t[:], AF.Relu)
                nc.sync.dma_start(out=out[b], in_=out_sb[:])
                del h2_sb[b], gf_tiles[b], g2_tiles[b], mtgt[b]
                del rft_tiles[b]
                for k in (0, 1):
                    for pb in {plan["pb_lo"][(b, k)], plan["pb_hi"][(b, k)]}:
                        if plan["last_user"][(k, pb)] == b:
                            del hp_sb[(k, pb)]

            # --- schedule: "heavy" passes = 16 k2gf + P0+P1 packed, with a
            # DEPTH-block k2 head start to absorb the W0/W1 DMA; packed
            # passes then alternate with the remaining k2 blocks.  builds
            # trail their k2gf block by one heavy pass; stage3(b) is emitted
            # as soon as its packed dependencies have been emitted. ---
            heavy = [("k2", b) for b in range(DEPTH)]
            pq = []
            for i in range(max(P0, P1)):
                if i < P0:
                    pq.append((0, i))
                if i < P1:
                    pq.append((1, i))
            bq = list(range(DEPTH, NBLK))
            while pq or bq:
                if pq:
                    heavy.append(("pp", pq.pop(0)))
                if bq:
                    heavy.append(("k2", bq.pop(0)))

            emitted_pb = {0: 0, 1: 0}
            built = set()
            done_upto = [0]  # next stage3 block

            def try_stage3():
                while (done_upto[0] < NBLK and done_upto[0] in built
                       and all(plan["pb_hi"][(done_upto[0], k)] < emitted_pb[k]
                               for k in (0, 1))):
                    stage3(done_upto[0])
                    done_upto[0] += 1

            pending_build = []
            prefetch_rft = 4     # rft blocks already requested upfront
            prefetch_pp = 2      # packed tiles already requested (1 per k)

            for hi, (kind, arg) in enumerate(heavy):
                if kind == "k2":
                    k2gf_pass(arg)
                    pending_build.append(arg)
                else:
                    k, pb = arg
                    ppass(k, pb)
                    emitted_pb[k] = pb + 1
                # deadline-ordered late loads: rft4/5/6 recycle buffers of
                # blocks 0/1/2 (WAR resolves after their k2gf), then W0, then
                # prft10+W1 — strictly by first-use time
                if hi == 0:
                    _load_rft(4)
                    prefetch_rft = 5
                elif hi == 1:
                    _load_rft(5)
                    prefetch_rft = 6
                elif hi == 2:
                    _load_rft(6)
                    prefetch_rft = 7
                elif hi == 3:
                    _load_w(0)
                elif hi == 4:
                    _load_prft(1, 0)
                    _load_w(1)
                # prefetch DMA for upcoming heavy passes (~3 ahead)
                for j in range(hi + 1, min(hi + 4, len(heavy))):
                    kj, aj = heavy[j]
                    if kj == "k2" and aj >= prefetch_rft:
                        for bb in range(prefetch_rft, aj + 1):
                            _load_rft(bb)
                        prefetch_rft = aj + 1
                    if kj == "pp" and aj not in prft_tiles and aj[1] >= (
                            emitted_pb[aj[0]]):
                        if aj not in prft_tiles:
                            _load_prft(*aj)
                # builds trail by one heavy pass
                while len(pending_build) > 1:
                    bb = pending_build.pop(0)
                    build(bb)
                    built.add(bb)
                try_stage3()
            while pending_build:
                bb = pending_build.pop(0)
                build(bb)
                built.add(bb)
            try_stage3()
            assert done_upto[0] == NBLK, f"only {done_upto[0]} blocks done"

    nc.compile()
    return nc


def _host_prep(inputs, plan):
    rf = np.asarray(inputs["region_feats"], dtype=np.float32)
    W = np.asarray(inputs["W_conv"], dtype=np.float32)
    Wg = np.asarray(inputs["W_g"], dtype=np.float32)
    blab = np.asarray(inputs["b_lab"], dtype=np.float32)
    bglab = np.asarray(inputs["b_glab"], dtype=np.float32)
    rels = np.asarray(inputs["rels"])
    preds = np.asarray(inputs["pred_classes"])

    rels_r = rels.reshape(N_IMG, RPI, 3)[:, :NUM_REL].reshape(-1, 3)
    preds_r = preds.reshape(N_IMG, RPI)[:, :NUM_REL].reshape(-1)

    # RF^T tiles: rft_h[b, p, d*128+j] = RF[b*128+j, d*128+p]
    rft_h = np.ascontiguousarray(
        rf.T.reshape(32, 128, NBLK, 128).transpose(2, 1, 0, 3), dtype=BF
    ).reshape(NBLK, 128, 32 * 128)

    # packed RF^T tiles per (k, pb)
    P0, P1 = plan["P"][0], plan["P"][1]
    prft_h = np.zeros((P0 + P1, 128, 32 * 128), BF)
    prft8_h = np.zeros((P0, 128, 32 * 128), F8) if FP8K0 else None
    for k in (0, 1):
        for pb in range(plan["P"][k]):
            regs = np.asarray(plan["pb_regs"][k][pb], np.int64)
            m = len(regs)
            # [m, D] -> [D, m] -> [32, 128, m] (d, p, j) -> (p, d, j)
            t = rf[regs].T.reshape(32, 128, m).transpose(1, 0, 2)
            i = pb if k == 0 else P0 + pb
            prft_h[i].reshape(128, 32, 128)[:, :, :m] = t.astype(BF)
            if FP8K0 and k == 0:
                prft8_h[pb].reshape(128, 32, 128)[:, :, :m] = _to_f8(t * S_R)

    # W slices per core, k-order (2, 0, 1):
    # w_h[p, ((ki*32+d)*CW)+j] = W[d*128+p, korder[ki]*D + c*CW + j]
    Wr = W.reshape(32, 128, 3, NCORES, CW)[:, :, (2, 0, 1)]
    w_cores = [
        np.ascontiguousarray(Wr[:, :, :, c, :].transpose(1, 2, 0, 3),
                             dtype=BF).reshape(128, 3 * 32 * CW)
        for c in range(NCORES)
    ]
    if FP8K0:
        # k=0 W slice (index 1 in (2,0,1) order), chunk-major, fp8-scaled
        w08_cores = [
            _to_f8(Wr[:, :, 1, c, :].transpose(1, 0, 2).reshape(128, 32 * CW)
                   * S_W)
            for c in range(NCORES)
        ]
    wg_h = np.ascontiguousarray(
        Wg.reshape(32, 128, 3).transpose(1, 0, 2), dtype=BF
    ).reshape(128, 32 * 3)
    blab_cores = [
        np.ascontiguousarray(blab[:, c * CW:(c + 1) * CW], dtype=BF)
        for c in range(NCORES)
    ]
    bgb_h = np.ascontiguousarray(
        np.repeat(bglab.reshape(1, NPRED), 128, axis=0), dtype=BF)

    srct_h = np.zeros((128, NBLK * 2 * EPB), np.float32)
    srcoL_h = np.zeros((EPB, NBLK * 2 * 128), np.float32)
    srcoH_h = np.zeros((EPB, plan["n_cross"] * 128), np.float32)
    tgto_h = np.zeros((EPB, NBLK * 2 * 128), np.float32)
    p1h_h = np.zeros((EPB, NBLK * NPRED), np.float32)
    e = np.arange(EPB)
    for b in range(NBLK):
        eb = rels_r[b * EPB:(b + 1) * EPB]
        pb_ = preds_r[b * EPB:(b + 1) * EPB]
        s = eb[:, 1] - b * 128
        o = eb[:, 2] - b * 128
        # k=0: obj -> subj (src=o, tgt=s); k=1: subj -> obj (src=s, tgt=o)
        for k, (src_loc, tgt_loc) in enumerate(((o, s), (s, o))):
            # fp8 path: descale factor folded into the packed-src one-hots
            oneval = 1.0 / (S_R * S_W) if (FP8K0 and k == 0) else 1.0
            srct_h[src_loc, (b * 2 + k) * EPB + e] = 1.0
            tgto_h[e, (b * 2 + k) * 128 + tgt_loc] = 1.0
            pk = plan["pos"][k][src_loc + b * 128]
            lo = plan["pb_lo"][(b, k)]
            hi = plan["pb_hi"][(b, k)]
            mlo = (pk // 128) == lo
            srcoL_h[e[mlo], (b * 2 + k) * 128 + (pk[mlo] - lo * 128)] = oneval
            if hi > lo:
                ci = plan["cross_idx"][(b, k)]
                mhi = ~mlo
                srcoH_h[e[mhi], ci * 128 + (pk[mhi] - hi * 128)] = oneval
        p1h_h[e, b * NPRED + pb_] = 1.0
    p1hs_h = np.zeros((128, NPRED), np.float32)
    p1hs_h[:, 0] = 1.0

    shared = {
        "rft": rft_h,
        "prft": prft_h,
        "wg": wg_h,
        "bgb": bgb_h,
        "srct": srct_h.astype(BF),
        "srcoL": srcoL_h.astype(BF),
        "srcoH": srcoH_h.astype(BF),
        "tgto": tgto_h.astype(BF),
        "p1h": p1h_h.astype(BF),
        "p1hs": p1hs_h.astype(BF),
        "ident": np.eye(128, dtype=np.float32).astype(BF),
    }
    if FP8K0:
        shared["prft8"] = prft8_h
    in_maps = []
    for c in range(NCORES):
        m = dict(shared)
        m["w"] = w_cores[c]
        m["blab"] = blab_cores[c]
        if FP8K0:
            m["w08"] = w08_cores[c]
        in_maps.append(m)
    return in_maps


def _rels_are_blocked(rels):
    """Check each image's relations reference only that image's regions."""
    rels = np.asarray(rels)
    if rels.shape != (N_IMG * RPI, 3):
        return False
    rels_r = rels.reshape(N_IMG, RPI, 3)[:, :NUM_REL]
    img = np.arange(N_IMG)[:, None]
    lo, hi = img * REG, (img + 1) * REG
    so = rels_r[:, :, 1:3]
    return bool(np.all((so >= lo[:, :, None]) & (so < hi[:, :, None])))


def _numpy_fallback(inputs):
    """Reference-equivalent host computation (only used if the per-image
    relation structure assumption is violated)."""
    rf = np.asarray(inputs["region_feats"], dtype=np.float32)
    W = np.asarray(inputs["W_conv"], dtype=np.float32)
    Wg = np.asarray(inputs["W_g"], dtype=np.float32)
    blab = np.asarray(inputs["b_lab"], dtype=np.float32)
    bglab = np.asarray(inputs["b_glab"], dtype=np.float32)
    rels = np.asarray(inputs["rels"])
    preds = np.asarray(inputs["pred_classes"])
    rels_r = rels.reshape(N_IMG, RPI, 3)[:, :NUM_REL].reshape(-1, 3)
    preds_r = preds.reshape(N_IMG, RPI)[:, :NUM_REL].reshape(-1)
    nf = (rf @ W).reshape(-1, D)
    gfe = (rf @ Wg).reshape(-1)
    s, o = rels_r[:, 1], rels_r[:, 2]
    self_ids = np.arange(N)
    idx = np.concatenate([o * 3 + 0, s * 3 + 1, self_ids * 3 + 2])
    pr = np.concatenate([preds_r, preds_r, np.zeros(N, preds_r.dtype)])
    tgt = np.concatenate([s, o, self_ids])
    gate = 1.0 / (1.0 + np.exp(-(gfe[idx] + bglab[pr, 0])))
    msg = gate[:, None] * (nf[idx] + blab[pr])
    upd = np.zeros((N, D), np.float32)
    np.add.at(upd, tgt, msg)
    return np.maximum(upd, 0.0)


def _emulate(inputs, plan, in_maps):
    """Numpy emulation of the device program from the STAGED tensors —
    validates host prep + on-device algebra (f32, no bf16 rounding)."""
    def f32(x):
        return np.asarray(x, dtype=np.float32)

    out = np.empty((N, D), np.float32)
    for c in range(NCORES):
        m = {k: f32(v) for k, v in in_maps[c].items()}
        w_k = {kk: m["w"][:, ki * 32 * CW:(ki + 1) * 32 * CW]
               for ki, kk in enumerate((2, 0, 1))}

        def lhsT_to_rows(tile):  # [128, 32*128] -> [128 rows, 4096]
            return tile.reshape(128, 32, 128).transpose(2, 1, 0).reshape(128, D)

        def wmat(kk):  # [128, 32*CW] -> [4096, CW]
            return w_k[kk].reshape(128, 32, CW).transpose(1, 0, 2).reshape(D, CW)

        hp = {}
        for k in (0, 1):
            for pb in range(plan["P"][k]):
                i = pb if k == 0 else plan["P"][0] + pb
                hp[(k, pb)] = lhsT_to_rows(m["prft"][i]) @ wmat(k)
        for b in range(NBLK):
            rows = lhsT_to_rows(m["rft"][b])
            h2 = rows @ wmat(2)
            gf = rows @ (m["wg"].reshape(128, 32, 3).transpose(1, 0, 2)
                         .reshape(D, 3))
            g2 = 1 / (1 + np.exp(-(m["bgb"][:, 0] + gf[:, 2])))
            h2s = g2[:, None] * h2
            pgt = np.zeros((NPRED, 128), np.float32)
            pout = np.zeros((128, CW), np.float32)
            for k in (0, 1):
                sig = 1 / (1 + np.exp(-(m["bgb"] + gf[:, k:k + 1])))
                sl = slice((b * 2 + k) * EPB, (b * 2 + k + 1) * EPB)
                sl128 = slice((b * 2 + k) * 128, (b * 2 + k + 1) * 128)
                prg = m["srct"][:, sl].T @ sig
                pg = prg * m["p1h"][:, b * NPRED:(b + 1) * NPRED]
                pgt += pg.T @ m["tgto"][:, sl128]
                gcol = pg.sum(1)
                sides = [(plan["pb_lo"][(b, k)], m["srcoL"][:, sl128])]
                if plan["pb_hi"][(b, k)] > plan["pb_lo"][(b, k)]:
                    ci = plan["cross_idx"][(b, k)]
                    sides.append((plan["pb_hi"][(b, k)],
                                  m["srcoH"][:, ci * 128:(ci + 1) * 128]))
                for pb, srco in sides:
                    srcg = srco * gcol[:, None]
                    pmt = srcg.T @ m["tgto"][:, sl128]
                    pout += pmt.T @ hp[(k, pb)]
            pg2 = m["p1hs"] * g2[:, None]
            pgt += pg2.T
            pout += pgt.T @ m["blab"]
            out[b * 128:(b + 1) * 128, c * CW:(c + 1) * CW] = np.maximum(
                pout + h2s, 0.0)
    return out


def _run(inputs, trace=False):
    from concourse.bass_utils import run_bass_kernel_spmd

    rels = np.asarray(inputs["rels"])
    plan = _plan_packing(rels)
    key = rels.tobytes()
    if key not in _prog_cache:
        _prog_cache.clear()
        _prog_cache[key] = _build_program(plan)
    nc = _prog_cache[key]
    in_maps = _host_prep(inputs, plan)
    try:
        res = run_bass_kernel_spmd(nc, in_maps, core_ids=list(range(NCORES)),
                                   trace=trace)
    except Exception:
        # transient device errors (e.g. NRT_EXEC_UNIT_UNRECOVERABLE) have
        # been observed to clear on retry
        import time
        time.sleep(5)
        res = run_bass_kernel_spmd(nc, in_maps, core_ids=list(range(NCORES)),
                                   trace=trace)
    out = np.empty((N, D), np.float32)
    for c in range(NCORES):
        out[:, c * CW:(c + 1) * CW] = np.asarray(
            res.results[c]["out"], dtype=np.float32).reshape(N, CW)
    return out, res


def kernel(**inputs):
    if not _rels_are_blocked(inputs["rels"]):
        return _numpy_fallback(inputs)
    out, _ = _run(inputs, trace=False)
    return out
